# revision 1
# baseline (speedup 1.0000x reference)
"""ChebNet GNN forward on 8 Trainium2 NeuronCores — data-parallel over the 8 graphs.

The input graph is a structured 3D grid (orientation ring x spatial grid), so the
sparse ChebConv Laplacian becomes a 6-point stencil. Per ChebConv we evaluate the
K=6 Chebyshev sum with the Clenshaw recurrence:
    b_5 = c_5;  b_k = c_k + 2L b_{k+1} - b_{k+2};  out = c_0 + L b_1 - b_2
where c_k = z @ W_k. We actually produce q = 2*out; BatchNorm (applied with
eps' = 4*eps on q-statistics) absorbs the factor exactly; the final BN-less conv
applies 0.5 explicitly.

On-device layouts (per core = one graph):
  feat-major [d, N]  for conv inputs z (PE matmul contraction on features)
  node-major [128, T*dout] for Clenshaw states (tile t = 128 consecutive nodes)
Lap terms: x/y-neighbor stencil -> per-tile banded 128x128 matrices on the PE
(c_k and the in-tile/cross-tile products accumulate in PSUM); the orientation
ring (+-tiles_per_layer with wrap) runs on the DVE with compact per-node weights
broadcast along the feature axis via stride-0 APs. BN statistics are AllReduced
across the 8 cores.
"""

import numpy as np
import ml_dtypes

from concourse import bass, bacc, tile, mybir
from concourse.bass_utils import run_bass_kernel_spmd

BF16 = mybir.dt.bfloat16
F32 = mybir.dt.float32
AF = mybir.ActivationFunctionType
OP = mybir.AluOpType

B, S, L = 8, 64, 6
K = 6
IN_D, HID, OUT_D = 3, 128, 10
EPS2 = 4e-5
N_CORES = 8
LEV_S = [S, S // 2, S // 4]
NPG = [L * s * s for s in LEV_S]          # nodes per graph per level
TILES = [n // 128 for n in NPG]           # 192, 48, 12
TPL = [s * s // 128 for s in LEV_S]       # tiles per layer: 32, 8, 2
CONV_LEV = [0, 0, 1, 1, 2, 2]
CONV_DIN = [IN_D, HID, HID, HID, HID, HID]
CONV_DOUT = [HID, HID, HID, HID, HID, OUT_D]
MC_CHUNK = 8                               # tiles per streamed M/C chunk (levels 0-1)


def _bf(x):
    return np.asarray(x).astype(ml_dtypes.bfloat16)


# --------------------------------------------------------------------------
# host-side preprocessing (numpy)
# --------------------------------------------------------------------------

def parse_grid_weights(edge_index, edge_attr, s):
    src = edge_index[0].astype(np.int64)
    dst = edge_index[1].astype(np.int64)
    ea = np.asarray(edge_attr, np.float64)

    def coords(n):
        return n // (s * s * L), (n // (s * s)) % L, (n // s) % s, n % s

    bs, os_, ys, xs = coords(src)
    bd, od, yd, xd = coords(dst)
    g = {k: np.zeros((B, L, s, s), np.float64)
         for k in ("xf", "xb", "yf", "yb", "of", "ob")}
    same = bs == bd
    so = same & (os_ == od)
    m = so & (yd == ys) & (xd == xs + 1)
    np.add.at(g["xf"], (bs[m], os_[m], ys[m], xs[m]), ea[m])
    m = so & (yd == ys) & (xd == xs - 1)
    np.add.at(g["xb"], (bd[m], od[m], yd[m], xd[m]), ea[m])
    m = so & (xd == xs) & (yd == ys + 1)
    np.add.at(g["yf"], (bs[m], os_[m], ys[m], xs[m]), ea[m])
    m = so & (xd == xs) & (yd == ys - 1)
    np.add.at(g["yb"], (bd[m], od[m], yd[m], xd[m]), ea[m])
    m = same & (yd == ys) & (xd == xs) & (od == (os_ + 1) % L)
    np.add.at(g["of"], (bs[m], os_[m], ys[m], xs[m]), ea[m])
    m = same & (yd == ys) & (xd == xs) & (od == (os_ - 1) % L)
    np.add.at(g["ob"], (bd[m], od[m], yd[m], xd[m]), ea[m])
    return {k: v.astype(np.float32) for k, v in g.items()}


def build_level_mats(gb, s):
    """gb: one graph's grids [L,s,s]. Returns M [T,128,128], Cup [T,128,64|128],
    Cdn likewise, wof_c [128,T], wob_c [128,T] (all x2-baked)."""
    N = L * s * s
    T = N // 128
    R = 128 // s
    xf = gb["xf"].reshape(L * s, s)
    xb = gb["xb"].reshape(L * s, s)
    yf = gb["yf"].reshape(L * s, s)
    yb = gb["yb"].reshape(L * s, s)

    M = np.zeros((T, 128, 128), np.float32)
    Cup = np.zeros((T, 128, 128), np.float32)
    Cdn = np.zeros((T, 128, 128), np.float32)
    ar = np.arange(s - 1)
    ars = np.arange(s)
    for t in range(T):
        for r in range(R):
            row = t * R + r
            base = r * s
            M[t, base + ar, base + ar + 1] += 2 * xf[row, :-1]
            M[t, base + ar + 1, base + ar] += 2 * xb[row, :-1]
            if r + 1 < R:
                M[t, base + ars, base + s + ars] += 2 * yf[row]
                M[t, base + s + ars, base + ars] += 2 * yb[row]
        if t > 0:
            Cup[t, (R - 1) * s + ars, ars] = 2 * yf[(t - 1) * R + (R - 1)]
        if t + 1 < T:
            Cdn[t, ars, ars] = 2 * yb[t * R + (R - 1)]   # cols shifted to 0 (compact); device offsets out partitions
    wof_c = 2 * gb["of"].reshape(T, 128).T
    wob_c = 2 * gb["ob"].reshape(T, 128).T
    return (M, Cup, Cdn, wof_c.astype(np.float32), wob_c.astype(np.float32))


def pack_chunks(Mt, cs, ncols):
    """[T, 128, ncols] -> [nchunks, 128, cs*ncols] partition-major chunks."""
    T = Mt.shape[0]
    nch = (T + cs - 1) // cs
    out = np.zeros((nch, 128, cs * ncols), np.float32)
    for g in range(nch):
        blk = Mt[g * cs:(g + 1) * cs, :, :ncols]          # [<=cs, 128, ncols]
        n = blk.shape[0]
        out[g, :, :n * ncols] = blk.transpose(1, 0, 2).reshape(128, n * ncols)
    return out


def host_preprocess(inputs):
    """Returns list of 8 per-core input dicts + shared shapes info."""
    x = np.asarray(inputs["x"], np.float32)
    per_core = [dict() for _ in range(N_CORES)]
    for b in range(N_CORES):
        per_core[b]["xT"] = _bf(x.reshape(B, NPG[0], IN_D)[b].T.copy())

    for lev, s in enumerate(LEV_S):
        g = parse_grid_weights(np.asarray(inputs[f"edge_index{lev+1}"]),
                               np.asarray(inputs[f"edge_attr{lev+1}"]), s)
        compact = lev < 2
        ncol_c = s if compact else 128
        cs = MC_CHUNK if compact else TILES[lev]
        for b in range(N_CORES):
            gb = {k: v[b] for k, v in g.items()}
            M, Cup, Cdn, wof, wob = build_level_mats(gb, s)
            if not compact:
                # dense Cdn: move cols back to natural position (R-1)*s..127
                R = 128 // s
                Cd2 = np.zeros_like(Cdn)
                Cd2[:, :, (R - 1) * s:] = Cdn[:, :, :s]
                Cdn = Cd2
            per_core[b][f"M{lev}"] = _bf(pack_chunks(M, cs, 128))
            per_core[b][f"Cup{lev}"] = _bf(pack_chunks(Cup, cs, ncol_c))
            per_core[b][f"Cdn{lev}"] = _bf(pack_chunks(Cdn, cs, ncol_c))
            per_core[b][f"wo{lev}"] = _bf(np.concatenate([wof, wob], axis=1))

    for i in range(6):
        Wk = np.asarray(inputs[f"W{i+1}"], np.float32)       # [K, din, dout]
        Wcat = np.concatenate([Wk[k] for k in range(K)], axis=1)  # [din, K*dout]
        for b in range(N_CORES):
            per_core[b][f"Wc{i}"] = _bf(Wcat)
    gam = np.stack([np.asarray(inputs[f"gamma{i+1}"], np.float32)
                    for i in range(5)], axis=1)              # [128, 5]
    bet = np.stack([np.asarray(inputs[f"beta{i+1}"], np.float32)
                    for i in range(5)], axis=1)
    ident = np.eye(128, dtype=np.float32)
    for b in range(N_CORES):
        per_core[b]["gam"] = gam
        per_core[b]["bet"] = bet
        per_core[b]["ident"] = _bf(ident)
    return per_core


# --------------------------------------------------------------------------
# device kernel builder
# --------------------------------------------------------------------------

def wrap_ranges(t0, nt, T):
    """[(src_start, dst_offset, n), ...] for tiles (t0..t0+nt) mod T."""
    out = []
    done = 0
    while done < nt:
        s0 = (t0 + done) % T
        n = min(nt - done, T - s0)
        out.append((s0, done, n))
        done += n
    return out


def build_bass(debug_stop=None):
    nc = bacc.Bacc("TRN2", target_bir_lowering=False, debug=False,
                   num_devices=N_CORES)

    # ---- dram parameters
    dri = {}

    def din(name, shape, dt):
        dri[name] = nc.dram_tensor(name, shape, dt, kind="ExternalInput").ap()

    din("xT", [IN_D, NPG[0]], BF16)
    for lev in range(3):
        T = TILES[lev]
        cs = MC_CHUNK if lev < 2 else T
        nch = (T + cs - 1) // cs
        ncol_c = LEV_S[lev] if lev < 2 else 128
        din(f"M{lev}", [nch, 128, cs * 128], BF16)
        din(f"Cup{lev}", [nch, 128, cs * ncol_c], BF16)
        din(f"Cdn{lev}", [nch, 128, cs * ncol_c], BF16)
        din(f"wo{lev}", [128, 2 * T], BF16)
    din("Wc0", [IN_D, K * HID], BF16)
    for i in range(1, 5):
        din(f"Wc{i}", [HID, K * HID], BF16)
    din("Wc5", [HID, K * OUT_D], BF16)
    din("gam", [128, 5], F32)
    din("bet", [128, 5], F32)
    din("ident", [128, 128], BF16)
    out_ap = nc.dram_tensor("out", [1, OUT_D], F32, kind="ExternalOutput").ap()
    dbg_ap = (nc.dram_tensor("dbg", [128, NPG[0]], BF16, kind="ExternalOutput").ap()
              if debug_stop is not None else None)

    with tile.TileContext(nc) as tc:
        with (
            tc.tile_pool(name="big", bufs=1) as big,
            tc.tile_pool(name="wpool", bufs=1) as wpool,
            tc.tile_pool(name="mc", bufs=2) as mcp,
            tc.tile_pool(name="chk", bufs=2) as chk,
            tc.tile_pool(name="sm", bufs=1) as sm,
            tc.tile_pool(name="ps", bufs=3, space="PSUM") as psp,
            tc.tile_pool(name="ps1", bufs=1, space="PSUM") as psp1,
            tc.tile_pool(name="dram", bufs=1, space="DRAM") as drp,
        ):
            N1 = NPG[0]
            Z = big.tile([128, N1], BF16, tag="Z")
            BA = big.tile([128, N1], BF16, tag="BA")
            BB = big.tile([128, N1], BF16, tag="BB")
            nc.vector.memset(BA[:], 0.0)
            nc.vector.memset(BB[:], 0.0)

            # resident weights
            Wc = []
            for i in range(6):
                t = wpool.tile(list(dri[f"Wc{i}"].shape), BF16, tag=f"Wc{i}")
                nc.sync.dma_start(t[:], dri[f"Wc{i}"][:])
                Wc.append(t)
            gam = sm.tile([128, 5], F32, tag="gam")
            bet = sm.tile([128, 5], F32, tag="bet")
            ident = sm.tile([128, 128], BF16, tag="ident")
            nc.sync.dma_start(gam[:], dri["gam"][:])
            nc.sync.dma_start(bet[:], dri["bet"][:])
            nc.sync.dma_start(ident[:], dri["ident"][:])

            # resident M/C for levels 1,2 + o-weights for all levels
            resM = {}
            for lev in (2,):
                for nm_ in ("M", "Cup", "Cdn"):
                    sap = dri[f"{nm_}{lev}"]
                    t = wpool.tile([128, sap.shape[2]], BF16, tag=f"{nm_}{lev}")
                    nc.sync.dma_start(t[:], sap[0])
                    resM[(nm_, lev)] = t
            wo = {}
            for lev in range(3):
                t = wpool.tile([128, 2 * TILES[lev]], BF16, tag=f"wo{lev}")
                nc.sync.dma_start(t[:], dri[f"wo{lev}"][:])
                wo[lev] = t

            # BN collective bounce
            bn_in = drp.tile([1, 2 * HID], F32)
            bn_out = drp.tile([1, 2 * HID], F32)

            # stats / bn vectors
            SP = sm.tile([128, HID], F32, tag="SP")
            SQP = sm.tile([128, HID], F32, tag="SQP")
            R1 = sm.tile([128, HID], F32, tag="R1")
            ONES = sm.tile([128, 1], F32, tag="ONES")
            nc.vector.memset(ONES[:], 1.0)
            BN2 = sm.tile([1, 2 * HID], F32, tag="BN2")
            G2 = sm.tile([128, 2], F32, tag="G2")
            MEAN = sm.tile([128, 1], F32, tag="MEAN")
            VAR = sm.tile([128, 1], F32, tag="VAR")
            TMPV = sm.tile([128, 1], F32, tag="TMPV")
            Av = sm.tile([128, 1], F32, tag="Av")
            Cv = sm.tile([128, 1], F32, tag="Cv")

            def conv(ci):
                lev = CONV_LEV[ci]
                dinw, dout = CONV_DIN[ci], CONV_DOUT[ci]
                T = TILES[lev]
                tpl = TPL[lev]
                Ncols = T * dout
                compact = lev < 2
                cs = MC_CHUNK if compact else T
                ncol_c = LEV_S[lev] if compact else 128
                dcs = cs                           # tiles per DVE chunk
                zt = Z
                ndch = T // dcs
                b1, b2 = BA, BB                    # b1 = current b_{k+1}
                for k in range(5, -1, -1):
                    for c in range(ndch):
                        t0 = c * dcs
                        if ci == 0:
                            zch = mcp.tile([IN_D, dcs * 128], BF16, tag="zch")
                            nc.sync.dma_start(
                                zch[:], dri["xT"][:, t0 * 128:(t0 + dcs) * 128])
                        if k < 5:
                            if compact:
                                mt = mcp.tile([128, dcs * 128], BF16, tag="mt")
                                cu = mcp.tile([128, dcs * ncol_c], BF16, tag="cu")
                                cd = mcp.tile([128, dcs * ncol_c], BF16, tag="cd")
                                nc.sync.dma_start(mt[:], dri[f"M{lev}"][c])
                                nc.sync.dma_start(cu[:], dri[f"Cup{lev}"][c])
                                nc.sync.dma_start(cd[:], dri[f"Cdn{lev}"][c])
                            else:
                                mt = resM[("M", lev)]
                                cu = resM[("Cup", lev)]
                                cd = resM[("Cdn", lev)]
                        if k < 5:
                            ev = chk.tile([128, dcs * dout], BF16, tag="ev")
                        for gi in range(dcs // 4):
                            ps = psp.tile([128, 4 * dout], F32, tag="ps")
                            for ii in range(4):
                                t = t0 + gi * 4 + ii
                                tl = gi * 4 + ii
                                pslice = ps[:, ii * dout:(ii + 1) * dout]
                                wsl = Wc[ci][:, k * dout:(k + 1) * dout]
                                zsl = (zch[:, tl * 128:(tl + 1) * 128] if ci == 0
                                       else zt[:, t * 128:(t + 1) * 128])
                                mms = [dict(out=pslice, lhsT=zsl, rhs=wsl)]
                                if k == 0:
                                    mms.append(dict(out=pslice, lhsT=zsl, rhs=wsl))
                                if k < 5:
                                    if t > 0:
                                        mms.append(dict(
                                            out=pslice[0:ncol_c, :] if compact else pslice,
                                            lhsT=cu[:, tl * ncol_c:(tl + 1) * ncol_c],
                                            rhs=b1[:, (t - 1) * dout:t * dout]))
                                    if t + 1 < T:
                                        if compact:
                                            mms.append(dict(
                                                out=pslice[128 - ncol_c:128, :],
                                                lhsT=cd[:, tl * ncol_c:(tl + 1) * ncol_c],
                                                rhs=b1[:, (t + 1) * dout:(t + 2) * dout],
                                                tile_position=(0, 128 - ncol_c)))
                                        else:
                                            mms.append(dict(
                                                out=pslice,
                                                lhsT=cd[:, tl * 128:(tl + 1) * 128],
                                                rhs=b1[:, (t + 1) * dout:(t + 2) * dout]))
                                    # full-partition M last so the group stop
                                    # covers every partition of the zero region
                                    mms.append(dict(
                                        out=pslice,
                                        lhsT=mt[:, tl * 128:(tl + 1) * 128],
                                        rhs=b1[:, t * dout:(t + 1) * dout]))
                                for mi, mm in enumerate(mms):
                                    nc.tensor.matmul(
                                        mm["out"], mm["lhsT"], mm["rhs"],
                                        start=(mi == 0), stop=(mi == len(mms) - 1),
                                        tile_position=mm.get("tile_position"))
                            if k == 5:
                                nc.scalar.copy(
                                    b2[:, (t0 + gi * 4) * dout:(t0 + gi * 4 + 4) * dout],
                                    ps[:])
                            else:
                                nc.scalar.copy(
                                    ev[:, gi * 4 * dout:(gi + 1) * 4 * dout], ps[:])
                        if k == 5:
                            continue
                        # DVE tail for this chunk -> write into b2 cols
                        cc = dcs * dout
                        c0 = t0 * dout
                        bslice = b2[:, c0:c0 + cc]
                        sc = 0.0 if k == 4 else (-2.0 if k == 0 else -1.0)
                        nc.vector.scalar_tensor_tensor(
                            bslice, bslice, sc, ev[:], OP.mult, OP.add)
                        # o-forward: dest tiles (t0..) <- src tiles -tpl
                        u1 = chk.tile([128, dcs * dout], BF16, tag="u1")
                        for (s0, doff, n) in wrap_ranges(t0 - tpl, dcs, T):
                            wv = wo[lev][:, s0:s0 + n]
                            nc.vector.tensor_tensor(
                                u1[:, doff * dout:(doff + n) * dout]
                                  .rearrange("p (t d) -> p t d", t=n),
                                b1[:, s0 * dout:(s0 + n) * dout]
                                  .rearrange("p (t d) -> p t d", t=n),
                                wv[:, :, None].broadcast_to([128, n, dout]),
                                OP.mult)
                        nc.vector.tensor_tensor(bslice, bslice, u1[:], OP.add)
                        # o-backward: weight at dest, value at +tpl
                        u2 = chk.tile([128, dcs * dout], BF16, tag="u2")
                        for (s0, doff, n) in wrap_ranges(t0 + tpl, dcs, T):
                            wv = wo[lev][:, T + t0 + doff:T + t0 + doff + n]
                            nc.vector.tensor_tensor(
                                u2[:, doff * dout:(doff + n) * dout]
                                  .rearrange("p (t d) -> p t d", t=n),
                                b1[:, s0 * dout:(s0 + n) * dout]
                                  .rearrange("p (t d) -> p t d", t=n),
                                wv[:, :, None].broadcast_to([128, n, dout]),
                                OP.mult)
                        nc.vector.tensor_tensor(bslice, bslice, u2[:], OP.add)
                    b1, b2 = b2, b1
                    if (isinstance(debug_stop, tuple) and debug_stop[0] == "b"
                            and debug_stop[1] == ci and debug_stop[2] == k):
                        nc.sync.dma_start(dbg_ap[:, 0:Ncols], b1[:, 0:Ncols])
                # q = 2p now lives in b1 (cols [0, Ncols))
                Q = b1

                if ci < 5:
                    # ---- BN stats on q: per-feature sums over (partition, tile)
                    nch2 = max(1, Ncols // 2048)
                    cc = Ncols // nch2
                    nt_c = cc // dout
                    for c in range(nch2):
                        qs_ = Q[:, c * cc:(c + 1) * cc]
                        qv = qs_.rearrange("p (t d) -> p d t", t=nt_c)
                        nc.vector.tensor_reduce(R1[:, 0:dout], qv,
                                                mybir.AxisListType.X, OP.add)
                        if c == 0:
                            nc.vector.tensor_copy(SP[:, 0:dout], R1[:, 0:dout])
                        else:
                            nc.vector.tensor_tensor(SP[:, 0:dout], SP[:, 0:dout],
                                                    R1[:, 0:dout], OP.add)
                        trash = chk.tile([128, cc], BF16, tag="u1")
                        nc.vector.tensor_tensor(trash[:], qs_, qs_, OP.mult)
                        nc.vector.tensor_reduce(
                            R1[:, 0:dout],
                            trash[:].rearrange("p (t d) -> p d t", t=nt_c),
                            mybir.AxisListType.X, OP.add)
                        if c == 0:
                            nc.vector.tensor_copy(SQP[:, 0:dout], R1[:, 0:dout])
                        else:
                            nc.vector.tensor_tensor(SQP[:, 0:dout], SQP[:, 0:dout],
                                                    R1[:, 0:dout], OP.add)
                    bnp = psp1.tile([1, 2 * HID], F32, tag="bnps")
                    nc.tensor.matmul(bnp[:, 0:dout], ONES[:], SP[:, 0:dout],
                                     start=True, stop=False)
                    nc.tensor.matmul(bnp[:, HID:HID + dout], ONES[:],
                                     SQP[:, 0:dout], start=False, stop=True)
                    nc.scalar.copy(BN2[:], bnp[:])
                    nc.sync.dma_start(bn_in[:], BN2[:])
                    nc.gpsimd.collective_compute(
                        "AllReduce", OP.add,
                        replica_groups=[list(range(N_CORES))],
                        ins=[bn_in.opt()], outs=[bn_out.opt()])
                    nc.sync.dma_start(G2[:, 0:1], bn_out[0:1, 0:HID])
                    nc.sync.dma_start(G2[:, 1:2], bn_out[0:1, HID:2 * HID])
                    ntot = float(N_CORES * NPG[lev])
                    nc.vector.tensor_scalar_mul(MEAN[:], G2[:, 0:1], 1.0 / ntot)
                    nc.vector.tensor_scalar_mul(VAR[:], G2[:, 1:2], 1.0 / ntot)
                    nc.vector.tensor_tensor(TMPV[:], MEAN[:], MEAN[:], OP.mult)
                    nc.vector.tensor_tensor(VAR[:], VAR[:], TMPV[:], OP.subtract)
                    nc.vector.tensor_scalar_add(VAR[:], VAR[:], EPS2)
                    nc.scalar.sqrt(TMPV[:], VAR[:])
                    nc.vector.reciprocal(TMPV[:], TMPV[:])
                    nc.vector.tensor_tensor(Av[:], gam[:, ci:ci + 1], TMPV[:],
                                            OP.mult)
                    nc.vector.tensor_tensor(TMPV[:], Av[:], MEAN[:], OP.mult)
                    nc.vector.tensor_tensor(Cv[:], bet[:, ci:ci + 1], TMPV[:],
                                            OP.subtract)
                    if debug_stop == ("bn", ci):
                        BNDBG = sm.tile([128, 6], F32, tag="BNDBG")
                        nc.vector.tensor_copy(BNDBG[:, 0:1], G2[:, 0:1])
                        nc.vector.tensor_copy(BNDBG[:, 1:2], G2[:, 1:2])
                        nc.vector.tensor_copy(BNDBG[:, 2:3], MEAN[:])
                        nc.vector.tensor_copy(BNDBG[:, 3:4], VAR[:])
                        nc.vector.tensor_copy(BNDBG[:, 4:5], Av[:])
                        nc.vector.tensor_copy(BNDBG[:, 5:6], Cv[:])
                        BNB16 = sm.tile([128, 6], BF16, tag="BNB16")
                        nc.vector.tensor_copy(BNB16[:], BNDBG[:])
                        nc.sync.dma_start(dbg_ap[:, 0:6], BNB16[:])

                # ---- transpose to feat-major + fused BN-relu (or 0.5-relu)
                if ci < 5:
                    for gi in range(T // 4):
                        ps = psp.tile([128, 4 * 128], BF16, tag="tps")
                        for ii in range(4):
                            t = gi * 4 + ii
                            nc.tensor.transpose(
                                ps[:, ii * 128:(ii + 1) * 128],
                                Q[:, t * dout:(t + 1) * dout], ident[:])
                        nc.scalar.activation(
                            Z[:, gi * 512:(gi + 1) * 512], ps[:], AF.Relu,
                            bias=Cv[:], scale=Av[:])
                else:
                    Z6 = sm.tile([OUT_D, TILES[2] * 128], BF16, tag="Z6")
                    for gi in range(T // 4):
                        ps = psp.tile([128, 4 * 128], BF16, tag="tps")
                        for ii in range(4):
                            t = gi * 4 + ii
                            nc.tensor.transpose(
                                ps[0:OUT_D, ii * 128:(ii + 1) * 128],
                                Q[:, t * dout:(t + 1) * dout], ident[:])
                        nc.scalar.activation(
                            Z6[:, gi * 512:(gi + 1) * 512], ps[0:OUT_D, :],
                            AF.Relu, bias=0.0, scale=0.5)
                    return Z6

            def pool2x2(s, d=128):
                """Z [d, L*s*s] -> Z [d, L*(s/2)^2] via temp in BA."""
                n = L * s * s
                half = n // 2
                tmp = BA
                # x-pairs
                nc.vector.tensor_tensor(
                    tmp[0:d, 0:half],
                    Z[0:d, 0:n].rearrange("p (c two) -> p c two", two=2)[:, :, 0:1]
                      .rearrange("p c one -> p (c one)"),
                    Z[0:d, 0:n].rearrange("p (c two) -> p c two", two=2)[:, :, 1:2]
                      .rearrange("p c one -> p (c one)"),
                    OP.max)
                # y-pairs: cols (o, y, x2) with x2 = s/2
                x2 = s // 2
                v = tmp[0:d, 0:half].rearrange("p (o y x) -> p o y x", o=L, y=s)
                nc.vector.tensor_tensor(
                    Z[0:d, 0:half // 2].rearrange("p (o y x) -> p o y x",
                                                  o=L, y=s // 2),
                    v[:, :, 0::2, :], v[:, :, 1::2, :], OP.max)

            RES = sm.tile([1, OUT_D], F32, tag="RES")
            # ---------------- network ----------------
            def dbg_dump(si, buf, n):
                if debug_stop == si:
                    nc.sync.dma_start(dbg_ap[:, 0:n], buf[:, 0:n])

            if isinstance(debug_stop, tuple):
                dnum = -1
            else:
                dnum = debug_stop if isinstance(debug_stop, int) else 99
            conv(0)
            dbg_dump(0, Z, NPG[0])
            if dnum >= 1:
                conv(1)
                dbg_dump(1, Z, NPG[0])
            if dnum >= 2:
                pool2x2(S)
                dbg_dump(2, Z, NPG[1])
            if dnum >= 3:
                conv(2)
                dbg_dump(3, Z, NPG[1])
            if dnum >= 4:
                conv(3)
                dbg_dump(4, Z, NPG[1])
            if dnum >= 5:
                pool2x2(S // 2)
                dbg_dump(5, Z, NPG[2])
            if dnum >= 6:
                conv(4)
                dbg_dump(6, Z, NPG[2])
            Z6 = conv(5) if dnum >= 7 else None
            if Z6 is None:
                nc.vector.memset(RES[:], 0.0)
                nc.sync.dma_start(out_ap[:], RES[:])

            if Z6 is not None:
                s3 = S // 4
                n3 = L * s3 * s3
                P3 = sm.tile([OUT_D, n3 // 4], BF16, tag="P3")
                TMP3 = sm.tile([OUT_D, n3 // 2], BF16, tag="TMP3")
                nc.vector.tensor_tensor(
                    TMP3[:],
                    Z6[:].rearrange("p (c two) -> p c two", two=2)[:, :, 0:1]
                         .rearrange("p c one -> p (c one)"),
                    Z6[:].rearrange("p (c two) -> p c two", two=2)[:, :, 1:2]
                         .rearrange("p c one -> p (c one)"),
                    OP.max)
                v3 = TMP3[:].rearrange("p (o y x) -> p o y x", o=L, y=s3)
                nc.vector.tensor_tensor(
                    P3[:].rearrange("p (o y x) -> p o y x", o=L, y=s3 // 2),
                    v3[:, :, 0::2, :], v3[:, :, 1::2, :], OP.max)
                # orientation max over L slices of 64
                spp = (s3 // 2) * (s3 // 2)
                OM = sm.tile([OUT_D, spp], BF16, tag="OM")
                nc.vector.tensor_tensor(OM[:], P3[:, 0:spp], P3[:, spp:2 * spp],
                                        OP.max)
                for o in range(2, L):
                    nc.vector.tensor_tensor(OM[:], OM[:],
                                            P3[:, o * spp:(o + 1) * spp], OP.max)
                GV = sm.tile([OUT_D, 1], F32, tag="GV")
                nc.vector.tensor_reduce(GV[:], OM[:], mybir.AxisListType.X, OP.max)
                # -> [1, 10] via DRAM bounce
                gb_d = drp.tile([OUT_D, 1], F32)
                nc.sync.dma_start(gb_d[:], GV[:])
                GF = sm.tile([1, OUT_D], F32, tag="GF")
                nc.sync.dma_start(GF[:], gb_d[:].rearrange("a b -> b a"))
                M0 = sm.tile([1, 1], F32, tag="M0")
                nc.vector.tensor_reduce(M0[:], GF[:], mybir.AxisListType.X, OP.max)
                TD = sm.tile([1, OUT_D], F32, tag="TD")
                nc.vector.tensor_scalar(TD[:], GF[:], M0[:], None, OP.subtract)
                EX = sm.tile([1, OUT_D], F32, tag="EX")
                nc.scalar.activation(EX[:], TD[:], AF.Exp)
                SE = sm.tile([1, 1], F32, tag="SE")
                nc.vector.tensor_reduce(SE[:], EX[:], mybir.AxisListType.X, OP.add)
                LSE = sm.tile([1, 1], F32, tag="LSE")
                nc.scalar.activation(LSE[:], SE[:], AF.Ln)
                nc.vector.tensor_scalar(RES[:], TD[:], LSE[:], None, OP.subtract)
                nc.sync.dma_start(out_ap[:], RES[:])

    nc.compile()
    return nc


_CACHE = {}


def _get_nc():
    if "nc" not in _CACHE:
        _CACHE["nc"] = build_bass()
    return _CACHE["nc"]


def kernel(**inputs):
    nc = _get_nc()
    per_core = host_preprocess(inputs)
    res = run_bass_kernel_spmd(nc, per_core, list(range(N_CORES)))
    out = np.concatenate([res.results[c]["out"] for c in range(N_CORES)], axis=0)
    return out.astype(np.float32)



# revision 3
# speedup vs baseline: 24.2011x; 24.2011x over previous
"""ChebNet GNN forward on 8 Trainium2 NeuronCores — data-parallel over the 8 graphs.

The input graph is a structured 3D grid (orientation ring x spatial grid), so the
sparse ChebConv Laplacian becomes a 6-point stencil. Per ChebConv we evaluate the
K=6 Chebyshev sum with the Clenshaw recurrence:
    b_5 = c_5;  b_k = c_k + 2L b_{k+1} - b_{k+2};  out = c_0 + L b_1 - b_2
where c_k = z @ W_k. We actually produce q = 2*out; BatchNorm (applied with
eps' = 4*eps on q-statistics) absorbs the factor exactly; the final BN-less conv
applies 0.5 explicitly.

On-device layouts (per core = one graph):
  feat-major [d, N]  for conv inputs z (PE matmul contraction on features)
  node-major [128, T*dout] for Clenshaw states (tile t = 128 consecutive nodes)
Lap terms: x/y-neighbor stencil -> per-tile banded 128x128 matrices on the PE
(c_k and the in-tile/cross-tile products accumulate in PSUM); the orientation
ring (+-tiles_per_layer with wrap) runs on the DVE with compact per-node weights
broadcast along the feature axis via stride-0 APs. BN statistics are AllReduced
across the 8 cores.
"""

import numpy as np
import ml_dtypes

from concourse import bass, bacc, tile, mybir
from concourse.bass_utils import run_bass_kernel_spmd

BF16 = mybir.dt.bfloat16
F32 = mybir.dt.float32
AF = mybir.ActivationFunctionType
OP = mybir.AluOpType

B, S, L = 8, 64, 6
K = 6
IN_D, HID, OUT_D = 3, 128, 10
EPS2 = 4e-5
N_CORES = 8
LEV_S = [S, S // 2, S // 4]
NPG = [L * s * s for s in LEV_S]          # nodes per graph per level
TILES = [n // 128 for n in NPG]           # 192, 48, 12
TPL = [s * s // 128 for s in LEV_S]       # tiles per layer: 32, 8, 2
CONV_LEV = [0, 0, 1, 1, 2, 2]
CONV_DIN = [IN_D, HID, HID, HID, HID, HID]
CONV_DOUT = [HID, HID, HID, HID, HID, OUT_D]
MC_CHUNK = 8                               # tiles per streamed M/C chunk (levels 0-1)


def _bf(x):
    return np.asarray(x).astype(ml_dtypes.bfloat16)


# --------------------------------------------------------------------------
# host-side preprocessing (numpy)
# --------------------------------------------------------------------------

def parse_grid_weights(edge_index, edge_attr, s):
    src = edge_index[0].astype(np.int64)
    dst = edge_index[1].astype(np.int64)
    ea = np.asarray(edge_attr, np.float64)

    def coords(n):
        return n // (s * s * L), (n // (s * s)) % L, (n // s) % s, n % s

    bs, os_, ys, xs = coords(src)
    bd, od, yd, xd = coords(dst)
    g = {k: np.zeros((B, L, s, s), np.float64)
         for k in ("xf", "xb", "yf", "yb", "of", "ob")}
    same = bs == bd
    so = same & (os_ == od)
    m = so & (yd == ys) & (xd == xs + 1)
    np.add.at(g["xf"], (bs[m], os_[m], ys[m], xs[m]), ea[m])
    m = so & (yd == ys) & (xd == xs - 1)
    np.add.at(g["xb"], (bd[m], od[m], yd[m], xd[m]), ea[m])
    m = so & (xd == xs) & (yd == ys + 1)
    np.add.at(g["yf"], (bs[m], os_[m], ys[m], xs[m]), ea[m])
    m = so & (xd == xs) & (yd == ys - 1)
    np.add.at(g["yb"], (bd[m], od[m], yd[m], xd[m]), ea[m])
    m = same & (yd == ys) & (xd == xs) & (od == (os_ + 1) % L)
    np.add.at(g["of"], (bs[m], os_[m], ys[m], xs[m]), ea[m])
    m = same & (yd == ys) & (xd == xs) & (od == (os_ - 1) % L)
    np.add.at(g["ob"], (bd[m], od[m], yd[m], xd[m]), ea[m])
    return {k: v.astype(np.float32) for k, v in g.items()}


def build_level_mats(gb, s):
    """gb: one graph's grids [L,s,s]. Returns M [T,128,128], Cup [T,128,64|128],
    Cdn likewise, wof_c [128,T], wob_c [128,T] (all x2-baked)."""
    N = L * s * s
    T = N // 128
    R = 128 // s
    xf = gb["xf"].reshape(L * s, s)
    xb = gb["xb"].reshape(L * s, s)
    yf = gb["yf"].reshape(L * s, s)
    yb = gb["yb"].reshape(L * s, s)

    M = np.zeros((T, 128, 128), np.float32)
    Cup = np.zeros((T, 128, 128), np.float32)
    Cdn = np.zeros((T, 128, 128), np.float32)
    ar = np.arange(s - 1)
    ars = np.arange(s)
    for t in range(T):
        for r in range(R):
            row = t * R + r
            base = r * s
            M[t, base + ar, base + ar + 1] += 2 * xf[row, :-1]
            M[t, base + ar + 1, base + ar] += 2 * xb[row, :-1]
            if r + 1 < R:
                M[t, base + ars, base + s + ars] += 2 * yf[row]
                M[t, base + s + ars, base + ars] += 2 * yb[row]
        if t > 0:
            Cup[t, (R - 1) * s + ars, ars] = 2 * yf[(t - 1) * R + (R - 1)]
        if t + 1 < T:
            Cdn[t, ars, ars] = 2 * yb[t * R + (R - 1)]   # cols shifted to 0 (compact); device offsets out partitions
    wof_c = 2 * gb["of"].reshape(T, 128).T
    wob_c = 2 * gb["ob"].reshape(T, 128).T
    return (M, Cup, Cdn, wof_c.astype(np.float32), wob_c.astype(np.float32))


def pack_chunks(Mt, cs, ncols):
    """[T, 128, ncols] -> [nchunks, 128, cs*ncols] partition-major chunks."""
    T = Mt.shape[0]
    nch = (T + cs - 1) // cs
    out = np.zeros((nch, 128, cs * ncols), np.float32)
    for g in range(nch):
        blk = Mt[g * cs:(g + 1) * cs, :, :ncols]          # [<=cs, 128, ncols]
        n = blk.shape[0]
        out[g, :, :n * ncols] = blk.transpose(1, 0, 2).reshape(128, n * ncols)
    return out


def host_preprocess(inputs):
    """Returns list of 8 per-core input dicts + shared shapes info."""
    x = np.asarray(inputs["x"], np.float32)
    per_core = [dict() for _ in range(N_CORES)]
    for b in range(N_CORES):
        per_core[b]["xT"] = _bf(x.reshape(B, NPG[0], IN_D)[b].T.copy())

    for lev, s in enumerate(LEV_S):
        g = parse_grid_weights(np.asarray(inputs[f"edge_index{lev+1}"]),
                               np.asarray(inputs[f"edge_attr{lev+1}"]), s)
        compact = lev < 2
        ncol_c = s if compact else 128
        cs = MC_CHUNK if compact else TILES[lev]
        for b in range(N_CORES):
            gb = {k: v[b] for k, v in g.items()}
            M, Cup, Cdn, wof, wob = build_level_mats(gb, s)
            if not compact:
                # dense Cdn: move cols back to natural position (R-1)*s..127
                R = 128 // s
                Cd2 = np.zeros_like(Cdn)
                Cd2[:, :, (R - 1) * s:] = Cdn[:, :, :s]
                Cdn = Cd2
            per_core[b][f"M{lev}"] = _bf(pack_chunks(M, cs, 128))
            per_core[b][f"Cup{lev}"] = _bf(pack_chunks(Cup, cs, ncol_c))
            per_core[b][f"Cdn{lev}"] = _bf(pack_chunks(Cdn, cs, ncol_c))
            per_core[b][f"wo{lev}"] = _bf(np.concatenate([wof, wob], axis=1))

    for i in range(6):
        Wk = np.asarray(inputs[f"W{i+1}"], np.float32)       # [K, din, dout]
        Wcat = np.concatenate([Wk[k] for k in range(K)], axis=1)  # [din, K*dout]
        for b in range(N_CORES):
            per_core[b][f"Wc{i}"] = _bf(Wcat)
    gam = np.stack([np.asarray(inputs[f"gamma{i+1}"], np.float32)
                    for i in range(5)], axis=1)              # [128, 5]
    bet = np.stack([np.asarray(inputs[f"beta{i+1}"], np.float32)
                    for i in range(5)], axis=1)
    ident = np.eye(128, dtype=np.float32)
    for b in range(N_CORES):
        per_core[b]["gam"] = gam
        per_core[b]["bet"] = bet
        per_core[b]["ident"] = _bf(ident)
    return per_core


# --------------------------------------------------------------------------
# device kernel builder
# --------------------------------------------------------------------------

def wrap_ranges(t0, nt, T):
    """[(src_start, dst_offset, n), ...] for tiles (t0..t0+nt) mod T."""
    out = []
    done = 0
    while done < nt:
        s0 = (t0 + done) % T
        n = min(nt - done, T - s0)
        out.append((s0, done, n))
        done += n
    return out


def build_bass(debug_stop=None, prof_nocoll=False):
    nc = bacc.Bacc("TRN2", target_bir_lowering=False, debug=False,
                   num_devices=N_CORES)

    # ---- dram parameters
    dri = {}

    def din(name, shape, dt):
        dri[name] = nc.dram_tensor(name, shape, dt, kind="ExternalInput").ap()

    din("xT", [IN_D, NPG[0]], BF16)
    for lev in range(3):
        T = TILES[lev]
        cs = MC_CHUNK if lev < 2 else T
        nch = (T + cs - 1) // cs
        ncol_c = LEV_S[lev] if lev < 2 else 128
        din(f"M{lev}", [nch, 128, cs * 128], BF16)
        din(f"Cup{lev}", [nch, 128, cs * ncol_c], BF16)
        din(f"Cdn{lev}", [nch, 128, cs * ncol_c], BF16)
        din(f"wo{lev}", [128, 2 * T], BF16)
    din("Wc0", [IN_D, K * HID], BF16)
    for i in range(1, 5):
        din(f"Wc{i}", [HID, K * HID], BF16)
    din("Wc5", [HID, K * OUT_D], BF16)
    din("gam", [128, 5], F32)
    din("bet", [128, 5], F32)
    din("ident", [128, 128], BF16)
    out_ap = nc.dram_tensor("out", [1, OUT_D], F32, kind="ExternalOutput").ap()
    dbg_ap = (nc.dram_tensor("dbg", [128, NPG[0]], BF16, kind="ExternalOutput").ap()
              if debug_stop is not None else None)

    with tile.TileContext(nc) as tc:
        with (
            tc.tile_pool(name="big", bufs=1) as big,
            tc.tile_pool(name="wpool", bufs=1) as wpool,
            tc.tile_pool(name="mc", bufs=2) as mcp,
            tc.tile_pool(name="chk", bufs=2) as chk,
            tc.tile_pool(name="sm", bufs=1) as sm,
            tc.tile_pool(name="ps", bufs=3, space="PSUM") as psp,
            tc.tile_pool(name="ps1", bufs=1, space="PSUM") as psp1,
            tc.tile_pool(name="dram", bufs=1, space="DRAM") as drp,
        ):
            N1 = NPG[0]
            Z = big.tile([128, N1], BF16, tag="Z")
            BA = big.tile([128, N1], BF16, tag="BA")
            BB = big.tile([128, N1], BF16, tag="BB")
            nc.vector.memset(BA[:], 0.0)
            nc.vector.memset(BB[:], 0.0)

            # resident weights
            Wc = []
            for i in range(6):
                t = wpool.tile(list(dri[f"Wc{i}"].shape), BF16, tag=f"Wc{i}")
                nc.sync.dma_start(t[:], dri[f"Wc{i}"][:])
                Wc.append(t)
            gam = sm.tile([128, 5], F32, tag="gam")
            bet = sm.tile([128, 5], F32, tag="bet")
            ident = sm.tile([128, 128], BF16, tag="ident")
            nc.sync.dma_start(gam[:], dri["gam"][:])
            nc.sync.dma_start(bet[:], dri["bet"][:])
            nc.sync.dma_start(ident[:], dri["ident"][:])

            # resident M/C for levels 1,2 + o-weights for all levels
            resM = {}
            for lev in (2,):
                for nm_ in ("M", "Cup", "Cdn"):
                    sap = dri[f"{nm_}{lev}"]
                    t = wpool.tile([128, sap.shape[2]], BF16, tag=f"{nm_}{lev}")
                    nc.sync.dma_start(t[:], sap[0])
                    resM[(nm_, lev)] = t
            wo = {}
            for lev in range(3):
                t = wpool.tile([128, 2 * TILES[lev]], BF16, tag=f"wo{lev}")
                nc.sync.dma_start(t[:], dri[f"wo{lev}"][:])
                wo[lev] = t

            # BN collective bounce
            bn_in = drp.tile([1, 2 * HID], F32)
            bn_out = drp.tile([1, 2 * HID], F32)

            # stats / bn vectors
            SP = sm.tile([128, HID], F32, tag="SP")
            SQP = sm.tile([128, HID], F32, tag="SQP")
            R1 = sm.tile([128, HID], F32, tag="R1")
            ONES = sm.tile([128, 1], F32, tag="ONES")
            nc.vector.memset(ONES[:], 1.0)
            BN2 = sm.tile([1, 2 * HID], F32, tag="BN2")
            G2 = sm.tile([128, 2], F32, tag="G2")
            MEAN = sm.tile([128, 1], F32, tag="MEAN")
            VAR = sm.tile([128, 1], F32, tag="VAR")
            TMPV = sm.tile([128, 1], F32, tag="TMPV")
            Av = sm.tile([128, 1], F32, tag="Av")
            Cv = sm.tile([128, 1], F32, tag="Cv")

            def conv(ci):
                lev = CONV_LEV[ci]
                dinw, dout = CONV_DIN[ci], CONV_DOUT[ci]
                T = TILES[lev]
                tpl = TPL[lev]
                Ncols = T * dout
                compact = lev < 2
                cs = MC_CHUNK if compact else T
                ncol_c = LEV_S[lev] if compact else 128
                dcs = cs                           # tiles per DVE chunk
                zt = Z
                ndch = T // dcs
                b1, b2 = BA, BB                    # b1 = current b_{k+1}
                for k in range(5, -1, -1):
                    for c in range(ndch):
                        t0 = c * dcs
                        if ci == 0:
                            zch = mcp.tile([IN_D, dcs * 128], BF16, tag="zch")
                            nc.sync.dma_start(
                                zch[:], dri["xT"][:, t0 * 128:(t0 + dcs) * 128])
                        if k < 5:
                            if compact:
                                mt = mcp.tile([128, dcs * 128], BF16, tag="mt")
                                cu = mcp.tile([128, dcs * ncol_c], BF16, tag="cu")
                                cd = mcp.tile([128, dcs * ncol_c], BF16, tag="cd")
                                nc.sync.dma_start(mt[:], dri[f"M{lev}"][c])
                                nc.sync.dma_start(cu[:], dri[f"Cup{lev}"][c])
                                nc.sync.dma_start(cd[:], dri[f"Cdn{lev}"][c])
                            else:
                                mt = resM[("M", lev)]
                                cu = resM[("Cup", lev)]
                                cd = resM[("Cdn", lev)]
                        if k < 5:
                            ev = chk.tile([128, dcs * dout], BF16, tag="ev")
                        for gi in range(dcs // 4):
                            ps = psp.tile([128, 4 * dout], F32, tag="ps")
                            for ii in range(4):
                                t = t0 + gi * 4 + ii
                                tl = gi * 4 + ii
                                pslice = ps[:, ii * dout:(ii + 1) * dout]
                                wsl = Wc[ci][:, k * dout:(k + 1) * dout]
                                zsl = (zch[:, tl * 128:(tl + 1) * 128] if ci == 0
                                       else zt[:, t * 128:(t + 1) * 128])
                                mms = [dict(out=pslice, lhsT=zsl, rhs=wsl)]
                                if k == 0:
                                    mms.append(dict(out=pslice, lhsT=zsl, rhs=wsl))
                                if k < 5:
                                    if t > 0:
                                        mms.append(dict(
                                            out=pslice[0:ncol_c, :] if compact else pslice,
                                            lhsT=cu[:, tl * ncol_c:(tl + 1) * ncol_c],
                                            rhs=b1[:, (t - 1) * dout:t * dout]))
                                    if t + 1 < T:
                                        if compact:
                                            mms.append(dict(
                                                out=pslice[128 - ncol_c:128, :],
                                                lhsT=cd[:, tl * ncol_c:(tl + 1) * ncol_c],
                                                rhs=b1[:, (t + 1) * dout:(t + 2) * dout],
                                                tile_position=(0, 128 - ncol_c)))
                                        else:
                                            mms.append(dict(
                                                out=pslice,
                                                lhsT=cd[:, tl * 128:(tl + 1) * 128],
                                                rhs=b1[:, (t + 1) * dout:(t + 2) * dout]))
                                    # full-partition M last so the group stop
                                    # covers every partition of the zero region
                                    mms.append(dict(
                                        out=pslice,
                                        lhsT=mt[:, tl * 128:(tl + 1) * 128],
                                        rhs=b1[:, t * dout:(t + 1) * dout]))
                                for mi, mm in enumerate(mms):
                                    nc.tensor.matmul(
                                        mm["out"], mm["lhsT"], mm["rhs"],
                                        start=(mi == 0), stop=(mi == len(mms) - 1),
                                        tile_position=mm.get("tile_position"))
                            if k == 5:
                                nc.scalar.copy(
                                    b2[:, (t0 + gi * 4) * dout:(t0 + gi * 4 + 4) * dout],
                                    ps[:])
                            else:
                                nc.scalar.copy(
                                    ev[:, gi * 4 * dout:(gi + 1) * 4 * dout], ps[:])
                        if k == 5:
                            continue
                        # DVE tail for this chunk -> write into b2 cols
                        cc = dcs * dout
                        c0 = t0 * dout
                        bslice = b2[:, c0:c0 + cc]
                        sc = 0.0 if k == 4 else (-2.0 if k == 0 else -1.0)
                        nc.vector.scalar_tensor_tensor(
                            bslice, bslice, sc, ev[:], OP.mult, OP.add)
                        # o-forward: dest tiles (t0..) <- src tiles -tpl
                        u1 = chk.tile([128, dcs * dout], BF16, tag="u1")
                        for (s0, doff, n) in wrap_ranges(t0 - tpl, dcs, T):
                            wv = wo[lev][:, s0:s0 + n]
                            nc.vector.tensor_tensor(
                                u1[:, doff * dout:(doff + n) * dout]
                                  .rearrange("p (t d) -> p t d", t=n),
                                b1[:, s0 * dout:(s0 + n) * dout]
                                  .rearrange("p (t d) -> p t d", t=n),
                                wv[:, :, None].broadcast_to([128, n, dout]),
                                OP.mult)
                        nc.vector.tensor_tensor(bslice, bslice, u1[:], OP.add)
                        # o-backward: weight at dest, value at +tpl
                        u2 = chk.tile([128, dcs * dout], BF16, tag="u2")
                        for (s0, doff, n) in wrap_ranges(t0 + tpl, dcs, T):
                            wv = wo[lev][:, T + t0 + doff:T + t0 + doff + n]
                            nc.vector.tensor_tensor(
                                u2[:, doff * dout:(doff + n) * dout]
                                  .rearrange("p (t d) -> p t d", t=n),
                                b1[:, s0 * dout:(s0 + n) * dout]
                                  .rearrange("p (t d) -> p t d", t=n),
                                wv[:, :, None].broadcast_to([128, n, dout]),
                                OP.mult)
                        nc.vector.tensor_tensor(bslice, bslice, u2[:], OP.add)
                    b1, b2 = b2, b1
                    if (isinstance(debug_stop, tuple) and debug_stop[0] == "b"
                            and debug_stop[1] == ci and debug_stop[2] == k):
                        nc.sync.dma_start(dbg_ap[:, 0:Ncols], b1[:, 0:Ncols])
                # q = 2p now lives in b1 (cols [0, Ncols))
                Q = b1

                if ci < 5:
                    # ---- BN stats on q: per-feature sums over (partition, tile)
                    nch2 = max(1, Ncols // 2048)
                    cc = Ncols // nch2
                    nt_c = cc // dout
                    for c in range(nch2):
                        qs_ = Q[:, c * cc:(c + 1) * cc]
                        qv = qs_.rearrange("p (t d) -> p d t", t=nt_c)
                        nc.vector.tensor_reduce(R1[:, 0:dout], qv,
                                                mybir.AxisListType.X, OP.add)
                        if c == 0:
                            nc.vector.tensor_copy(SP[:, 0:dout], R1[:, 0:dout])
                        else:
                            nc.vector.tensor_tensor(SP[:, 0:dout], SP[:, 0:dout],
                                                    R1[:, 0:dout], OP.add)
                        trash = chk.tile([128, cc], BF16, tag="u1")
                        nc.vector.tensor_tensor(trash[:], qs_, qs_, OP.mult)
                        nc.vector.tensor_reduce(
                            R1[:, 0:dout],
                            trash[:].rearrange("p (t d) -> p d t", t=nt_c),
                            mybir.AxisListType.X, OP.add)
                        if c == 0:
                            nc.vector.tensor_copy(SQP[:, 0:dout], R1[:, 0:dout])
                        else:
                            nc.vector.tensor_tensor(SQP[:, 0:dout], SQP[:, 0:dout],
                                                    R1[:, 0:dout], OP.add)
                    bnp = psp1.tile([1, 2 * HID], F32, tag="bnps")
                    nc.tensor.matmul(bnp[:, 0:dout], ONES[:], SP[:, 0:dout],
                                     start=True, stop=False)
                    nc.tensor.matmul(bnp[:, HID:HID + dout], ONES[:],
                                     SQP[:, 0:dout], start=False, stop=True)
                    nc.scalar.copy(BN2[:], bnp[:])
                    nc.sync.dma_start(bn_in[:], BN2[:])
                    if prof_nocoll:
                        nc.sync.dma_start(bn_out[:], bn_in[:])
                    else:
                        nc.gpsimd.collective_compute(
                            "AllReduce", OP.add,
                            replica_groups=[list(range(N_CORES))],
                            ins=[bn_in.opt()], outs=[bn_out.opt()])
                    nc.sync.dma_start(G2[:, 0:1], bn_out[0:1, 0:HID])
                    nc.sync.dma_start(G2[:, 1:2], bn_out[0:1, HID:2 * HID])
                    ntot = float(N_CORES * NPG[lev])
                    nc.vector.tensor_scalar_mul(MEAN[:], G2[:, 0:1], 1.0 / ntot)
                    nc.vector.tensor_scalar_mul(VAR[:], G2[:, 1:2], 1.0 / ntot)
                    nc.vector.tensor_tensor(TMPV[:], MEAN[:], MEAN[:], OP.mult)
                    nc.vector.tensor_tensor(VAR[:], VAR[:], TMPV[:], OP.subtract)
                    nc.vector.tensor_scalar_add(VAR[:], VAR[:], EPS2)
                    nc.scalar.sqrt(TMPV[:], VAR[:])
                    nc.vector.reciprocal(TMPV[:], TMPV[:])
                    nc.vector.tensor_tensor(Av[:], gam[:, ci:ci + 1], TMPV[:],
                                            OP.mult)
                    nc.vector.tensor_tensor(TMPV[:], Av[:], MEAN[:], OP.mult)
                    nc.vector.tensor_tensor(Cv[:], bet[:, ci:ci + 1], TMPV[:],
                                            OP.subtract)
                    if debug_stop == ("bn", ci):
                        BNDBG = sm.tile([128, 6], F32, tag="BNDBG")
                        nc.vector.tensor_copy(BNDBG[:, 0:1], G2[:, 0:1])
                        nc.vector.tensor_copy(BNDBG[:, 1:2], G2[:, 1:2])
                        nc.vector.tensor_copy(BNDBG[:, 2:3], MEAN[:])
                        nc.vector.tensor_copy(BNDBG[:, 3:4], VAR[:])
                        nc.vector.tensor_copy(BNDBG[:, 4:5], Av[:])
                        nc.vector.tensor_copy(BNDBG[:, 5:6], Cv[:])
                        BNB16 = sm.tile([128, 6], BF16, tag="BNB16")
                        nc.vector.tensor_copy(BNB16[:], BNDBG[:])
                        nc.sync.dma_start(dbg_ap[:, 0:6], BNB16[:])

                # ---- transpose to feat-major + fused BN-relu (or 0.5-relu)
                if ci < 5:
                    for gi in range(T // 4):
                        ps = psp.tile([128, 4 * 128], BF16, tag="tps")
                        for ii in range(4):
                            t = gi * 4 + ii
                            nc.tensor.transpose(
                                ps[:, ii * 128:(ii + 1) * 128],
                                Q[:, t * dout:(t + 1) * dout], ident[:])
                        nc.scalar.activation(
                            Z[:, gi * 512:(gi + 1) * 512], ps[:], AF.Relu,
                            bias=Cv[:], scale=Av[:])
                else:
                    Z6 = sm.tile([OUT_D, TILES[2] * 128], BF16, tag="Z6")
                    for gi in range(T // 4):
                        ps = psp.tile([128, 4 * 128], BF16, tag="tps")
                        for ii in range(4):
                            t = gi * 4 + ii
                            nc.tensor.transpose(
                                ps[0:OUT_D, ii * 128:(ii + 1) * 128],
                                Q[:, t * dout:(t + 1) * dout], ident[:])
                        nc.scalar.activation(
                            Z6[:, gi * 512:(gi + 1) * 512], ps[0:OUT_D, :],
                            AF.Relu, bias=0.0, scale=0.5)
                    return Z6

            def pool2x2(s, d=128):
                """Z [d, L*s*s] -> Z [d, L*(s/2)^2] via temp in BA."""
                n = L * s * s
                half = n // 2
                tmp = BA
                # x-pairs
                nc.vector.tensor_tensor(
                    tmp[0:d, 0:half],
                    Z[0:d, 0:n].rearrange("p (c two) -> p c two", two=2)[:, :, 0:1]
                      .rearrange("p c one -> p (c one)"),
                    Z[0:d, 0:n].rearrange("p (c two) -> p c two", two=2)[:, :, 1:2]
                      .rearrange("p c one -> p (c one)"),
                    OP.max)
                # y-pairs: cols (o, y, x2) with x2 = s/2
                x2 = s // 2
                v = tmp[0:d, 0:half].rearrange("p (o y x) -> p o y x", o=L, y=s)
                nc.vector.tensor_tensor(
                    Z[0:d, 0:half // 2].rearrange("p (o y x) -> p o y x",
                                                  o=L, y=s // 2),
                    v[:, :, 0::2, :], v[:, :, 1::2, :], OP.max)

            RES = sm.tile([1, OUT_D], F32, tag="RES")
            # ---------------- network ----------------
            def dbg_dump(si, buf, n):
                if debug_stop == si:
                    nc.sync.dma_start(dbg_ap[:, 0:n], buf[:, 0:n])

            if isinstance(debug_stop, tuple):
                dnum = -1
            else:
                dnum = debug_stop if isinstance(debug_stop, int) else 99
            conv(0)
            dbg_dump(0, Z, NPG[0])
            if dnum >= 1:
                conv(1)
                dbg_dump(1, Z, NPG[0])
            if dnum >= 2:
                pool2x2(S)
                dbg_dump(2, Z, NPG[1])
            if dnum >= 3:
                conv(2)
                dbg_dump(3, Z, NPG[1])
            if dnum >= 4:
                conv(3)
                dbg_dump(4, Z, NPG[1])
            if dnum >= 5:
                pool2x2(S // 2)
                dbg_dump(5, Z, NPG[2])
            if dnum >= 6:
                conv(4)
                dbg_dump(6, Z, NPG[2])
            Z6 = conv(5) if dnum >= 7 else None
            if Z6 is None:
                nc.vector.memset(RES[:], 0.0)
                nc.sync.dma_start(out_ap[:], RES[:])

            if Z6 is not None:
                s3 = S // 4
                n3 = L * s3 * s3
                P3 = sm.tile([OUT_D, n3 // 4], BF16, tag="P3")
                TMP3 = sm.tile([OUT_D, n3 // 2], BF16, tag="TMP3")
                nc.vector.tensor_tensor(
                    TMP3[:],
                    Z6[:].rearrange("p (c two) -> p c two", two=2)[:, :, 0:1]
                         .rearrange("p c one -> p (c one)"),
                    Z6[:].rearrange("p (c two) -> p c two", two=2)[:, :, 1:2]
                         .rearrange("p c one -> p (c one)"),
                    OP.max)
                v3 = TMP3[:].rearrange("p (o y x) -> p o y x", o=L, y=s3)
                nc.vector.tensor_tensor(
                    P3[:].rearrange("p (o y x) -> p o y x", o=L, y=s3 // 2),
                    v3[:, :, 0::2, :], v3[:, :, 1::2, :], OP.max)
                # orientation max over L slices of 64
                spp = (s3 // 2) * (s3 // 2)
                OM = sm.tile([OUT_D, spp], BF16, tag="OM")
                nc.vector.tensor_tensor(OM[:], P3[:, 0:spp], P3[:, spp:2 * spp],
                                        OP.max)
                for o in range(2, L):
                    nc.vector.tensor_tensor(OM[:], OM[:],
                                            P3[:, o * spp:(o + 1) * spp], OP.max)
                GV = sm.tile([OUT_D, 1], F32, tag="GV")
                nc.vector.tensor_reduce(GV[:], OM[:], mybir.AxisListType.X, OP.max)
                # -> [1, 10] via DRAM bounce
                gb_d = drp.tile([OUT_D, 1], F32)
                nc.sync.dma_start(gb_d[:], GV[:])
                GF = sm.tile([1, OUT_D], F32, tag="GF")
                nc.sync.dma_start(GF[:], gb_d[:].rearrange("a b -> b a"))
                M0 = sm.tile([1, 1], F32, tag="M0")
                nc.vector.tensor_reduce(M0[:], GF[:], mybir.AxisListType.X, OP.max)
                TD = sm.tile([1, OUT_D], F32, tag="TD")
                nc.vector.tensor_scalar(TD[:], GF[:], M0[:], None, OP.subtract)
                EX = sm.tile([1, OUT_D], F32, tag="EX")
                nc.scalar.activation(EX[:], TD[:], AF.Exp)
                SE = sm.tile([1, 1], F32, tag="SE")
                nc.vector.tensor_reduce(SE[:], EX[:], mybir.AxisListType.X, OP.add)
                LSE = sm.tile([1, 1], F32, tag="LSE")
                nc.scalar.activation(LSE[:], SE[:], AF.Ln)
                nc.vector.tensor_scalar(RES[:], TD[:], LSE[:], None, OP.subtract)
                nc.sync.dma_start(out_ap[:], RES[:])

    nc.compile()
    return nc


_CACHE = {}


def _get_nc():
    if "nc" not in _CACHE:
        _CACHE["nc"] = build_bass()
    return _CACHE["nc"]


def kernel(**inputs):
    nc = _get_nc()
    per_core = host_preprocess(inputs)
    res = run_bass_kernel_spmd(nc, per_core, list(range(N_CORES)))
    out = np.concatenate([res.results[c]["out"] for c in range(N_CORES)], axis=0)
    return out.astype(np.float32)



# revision 27
# speedup vs baseline: 26.3666x; 1.0895x over previous
"""ChebNet GNN forward on 8 Trainium2 NeuronCores — data-parallel over the 8 graphs.

The input graph is a structured 3D grid (orientation ring x spatial grid), so the
sparse ChebConv Laplacian becomes a 6-point stencil. Per ChebConv we evaluate the
K=6 Chebyshev sum with the Clenshaw recurrence:
    b_5 = c_5;  b_k = c_k + 2L b_{k+1} - b_{k+2};  out = c_0 + L b_1 - b_2
where c_k = z @ W_k. We actually produce q = 2*out; BatchNorm (applied with
eps' = 4*eps on q-statistics) absorbs the factor exactly; the final BN-less conv
applies 0.5 explicitly.

On-device layouts (per core = one graph):
  feat-major [d, N]  for conv inputs z (PE matmul contraction on features)
  node-major [128, T*dout] for Clenshaw states (tile t = 128 consecutive nodes)
Lap terms: x/y-neighbor stencil -> per-tile banded 128x128 matrices on the PE
(c_k and the in-tile/cross-tile products accumulate in PSUM); the orientation
ring (+-tiles_per_layer with wrap) runs on the DVE with compact per-node weights
broadcast along the feature axis via stride-0 APs. BN statistics are AllReduced
across the 8 cores.
"""

import numpy as np
import ml_dtypes

from concourse import bass, bacc, tile, mybir
from concourse.bass_utils import run_bass_kernel_spmd

BF16 = mybir.dt.bfloat16
F32 = mybir.dt.float32
AF = mybir.ActivationFunctionType
OP = mybir.AluOpType

B, S, L = 8, 64, 6
K = 6
IN_D, HID, OUT_D = 3, 128, 10
EPS2 = 4e-5
N_CORES = 8
LEV_S = [S, S // 2, S // 4]
NPG = [L * s * s for s in LEV_S]          # nodes per graph per level
TILES = [n // 128 for n in NPG]           # 192, 48, 12
TPL = [s * s // 128 for s in LEV_S]       # tiles per layer: 32, 8, 2
CONV_LEV = [0, 0, 1, 1, 2, 2]
CONV_DIN = [IN_D, HID, HID, HID, HID, HID]
CONV_DOUT = [HID, HID, HID, HID, HID, OUT_D]
MC_CHUNK = 8                               # tiles per streamed M/C chunk (levels 0-1)


def _bf(x):
    return np.asarray(x).astype(ml_dtypes.bfloat16)


# --------------------------------------------------------------------------
# host-side preprocessing (numpy)
# --------------------------------------------------------------------------

def parse_grid_weights(edge_index, edge_attr, s):
    src = edge_index[0].astype(np.int64)
    dst = edge_index[1].astype(np.int64)
    ea = np.asarray(edge_attr, np.float64)

    def coords(n):
        return n // (s * s * L), (n // (s * s)) % L, (n // s) % s, n % s

    bs, os_, ys, xs = coords(src)
    bd, od, yd, xd = coords(dst)
    g = {k: np.zeros((B, L, s, s), np.float64)
         for k in ("xf", "xb", "yf", "yb", "of", "ob")}
    same = bs == bd
    so = same & (os_ == od)
    m = so & (yd == ys) & (xd == xs + 1)
    np.add.at(g["xf"], (bs[m], os_[m], ys[m], xs[m]), ea[m])
    m = so & (yd == ys) & (xd == xs - 1)
    np.add.at(g["xb"], (bd[m], od[m], yd[m], xd[m]), ea[m])
    m = so & (xd == xs) & (yd == ys + 1)
    np.add.at(g["yf"], (bs[m], os_[m], ys[m], xs[m]), ea[m])
    m = so & (xd == xs) & (yd == ys - 1)
    np.add.at(g["yb"], (bd[m], od[m], yd[m], xd[m]), ea[m])
    m = same & (yd == ys) & (xd == xs) & (od == (os_ + 1) % L)
    np.add.at(g["of"], (bs[m], os_[m], ys[m], xs[m]), ea[m])
    m = same & (yd == ys) & (xd == xs) & (od == (os_ - 1) % L)
    np.add.at(g["ob"], (bd[m], od[m], yd[m], xd[m]), ea[m])
    return {k: v.astype(np.float32) for k, v in g.items()}


def build_level_mats(gb, s):
    """gb: one graph's grids [L,s,s]. Returns M [T,128,128], Cup [T,128,64|128],
    Cdn likewise, Df/Db [T,128,128] o-ring diagonals (all x2-baked)."""
    N = L * s * s
    T = N // 128
    R = 128 // s
    tpl = s * s // 128
    xf = gb["xf"].reshape(L * s, s)
    xb = gb["xb"].reshape(L * s, s)
    yf = gb["yf"].reshape(L * s, s)
    yb = gb["yb"].reshape(L * s, s)

    M = np.zeros((T, 128, 128), np.float32)
    Cup = np.zeros((T, 128, 128), np.float32)
    Cdn = np.zeros((T, 128, 128), np.float32)
    ar = np.arange(s - 1)
    ars = np.arange(s)
    for t in range(T):
        for r in range(R):
            row = t * R + r
            base = r * s
            M[t, base + ar, base + ar + 1] += 2 * xf[row, :-1]
            M[t, base + ar + 1, base + ar] += 2 * xb[row, :-1]
            if r + 1 < R:
                M[t, base + ars, base + s + ars] += 2 * yf[row]
                M[t, base + s + ars, base + ars] += 2 * yb[row]
        if t > 0:
            Cup[t, (R - 1) * s + ars, ars] = 2 * yf[(t - 1) * R + (R - 1)]
        if t + 1 < T:
            Cdn[t, ars, ars] = 2 * yb[t * R + (R - 1)]   # cols shifted to 0 (compact); device offsets out partitions
    # o-ring as per-tile diagonal matrices: dest tile t gets
    #   Df[t] @ b[(t-tpl)%T]  (forward edge, weight indexed at source)
    #   Db[t] @ b[(t+tpl)%T]  (backward edge, weight indexed at dest)
    wof_c = 2 * gb["of"].reshape(T, 128)        # [T(tile), 128(node)]
    wob_c = 2 * gb["ob"].reshape(T, 128)
    ai = np.arange(128)
    Df = np.zeros((T, 128, 128), np.float32)
    Db = np.zeros((T, 128, 128), np.float32)
    srcf = (np.arange(T) - tpl) % T
    Df[np.arange(T)[:, None], ai, ai] = wof_c[srcf]
    Db[np.arange(T)[:, None], ai, ai] = wob_c
    return (M, Cup, Cdn, Df, Db)


def pack_chunks(Mt, cs, ncols):
    """[T, 128, ncols] -> [nchunks, 128, cs*ncols] partition-major chunks."""
    T = Mt.shape[0]
    nch = (T + cs - 1) // cs
    out = np.zeros((nch, 128, cs * ncols), np.float32)
    for g in range(nch):
        blk = Mt[g * cs:(g + 1) * cs, :, :ncols]          # [<=cs, 128, ncols]
        n = blk.shape[0]
        out[g, :, :n * ncols] = blk.transpose(1, 0, 2).reshape(128, n * ncols)
    return out


def host_preprocess(inputs):
    """Returns list of 8 per-core input dicts + shared shapes info."""
    x = np.asarray(inputs["x"], np.float32)
    per_core = [dict() for _ in range(N_CORES)]
    for b in range(N_CORES):
        per_core[b]["xT"] = _bf(x.reshape(B, NPG[0], IN_D)[b].T.copy())

    for lev, s in enumerate(LEV_S):
        g = parse_grid_weights(np.asarray(inputs[f"edge_index{lev+1}"]),
                               np.asarray(inputs[f"edge_attr{lev+1}"]), s)
        compact = lev < 2
        ncol_c = s if compact else 128
        cs = MC_CHUNK if compact else TILES[lev]
        for b in range(N_CORES):
            gb = {k: v[b] for k, v in g.items()}
            M, Cup, Cdn, Df, Db = build_level_mats(gb, s)
            if not compact:
                # dense Cdn: move cols back to natural position (R-1)*s..127
                R = 128 // s
                Cd2 = np.zeros_like(Cdn)
                Cd2[:, :, (R - 1) * s:] = Cdn[:, :, :s]
                Cdn = Cd2
            per_core[b][f"M{lev}"] = _bf(pack_chunks(M, cs, 128))
            per_core[b][f"Cup{lev}"] = _bf(pack_chunks(Cup, cs, ncol_c))
            per_core[b][f"Cdn{lev}"] = _bf(pack_chunks(Cdn, cs, ncol_c))
            if lev == 0:
                # o-ring on DVE for level 0: per-node weights [128, 2T]
                wof = 2 * gb["of"].reshape(-1, 128).T
                wob = 2 * gb["ob"].reshape(-1, 128).T
                per_core[b]["wo0"] = _bf(
                    np.concatenate([wof, wob], axis=1).astype(np.float32))
            else:
                per_core[b][f"Df{lev}"] = _bf(pack_chunks(Df, cs, 128))
                per_core[b][f"Db{lev}"] = _bf(pack_chunks(Db, cs, 128))

    for i in range(6):
        Wk = np.asarray(inputs[f"W{i+1}"], np.float32)       # [K, din, dout]
        Wcat = np.concatenate([Wk[k] for k in range(K)], axis=1)  # [din, K*dout]
        for b in range(N_CORES):
            per_core[b][f"Wc{i}"] = _bf(Wcat)
    gam = np.stack([np.asarray(inputs[f"gamma{i+1}"], np.float32)
                    for i in range(5)], axis=1)              # [128, 5]
    bet = np.stack([np.asarray(inputs[f"beta{i+1}"], np.float32)
                    for i in range(5)], axis=1)
    ident = np.eye(128, dtype=np.float32)
    for b in range(N_CORES):
        per_core[b]["gam"] = gam
        per_core[b]["bet"] = bet
        per_core[b]["ident"] = _bf(ident)
        per_core[b]["negI"] = _bf(-ident)
        per_core[b]["neg2I"] = _bf(-2.0 * ident)
    return per_core


# --------------------------------------------------------------------------
# device kernel builder
# --------------------------------------------------------------------------

def wrap_ranges(t0, nt, T):
    """[(src_start, dst_offset, n), ...] for tiles (t0..t0+nt) mod T."""
    out = []
    done = 0
    while done < nt:
        s0 = (t0 + done) % T
        n = min(nt - done, T - s0)
        out.append((s0, done, n))
        done += n
    return out


def build_bass(debug_stop=None, prof_nocoll=False):
    nc = bacc.Bacc("TRN2", target_bir_lowering=False, debug=False,
                   num_devices=N_CORES)

    # ---- dram parameters
    dri = {}

    def din(name, shape, dt):
        dri[name] = nc.dram_tensor(name, shape, dt, kind="ExternalInput").ap()

    din("xT", [IN_D, NPG[0]], BF16)
    for lev in range(3):
        T = TILES[lev]
        cs = MC_CHUNK if lev < 2 else T
        nch = (T + cs - 1) // cs
        ncol_c = LEV_S[lev] if lev < 2 else 128
        din(f"M{lev}", [nch, 128, cs * 128], BF16)
        din(f"Cup{lev}", [nch, 128, cs * ncol_c], BF16)
        din(f"Cdn{lev}", [nch, 128, cs * ncol_c], BF16)
        if lev == 0:
            din("wo0", [128, 2 * T], BF16)
        else:
            din(f"Df{lev}", [nch, 128, cs * 128], BF16)
            din(f"Db{lev}", [nch, 128, cs * 128], BF16)
    din("Wc0", [IN_D, K * HID], BF16)
    for i in range(1, 5):
        din(f"Wc{i}", [HID, K * HID], BF16)
    din("Wc5", [HID, K * OUT_D], BF16)
    din("gam", [128, 5], F32)
    din("bet", [128, 5], F32)
    din("ident", [128, 128], BF16)
    din("negI", [128, 128], BF16)
    din("neg2I", [128, 128], BF16)
    out_ap = nc.dram_tensor("out", [1, OUT_D], F32, kind="ExternalOutput").ap()
    dbg_ap = (nc.dram_tensor("dbg", [128, NPG[0]], BF16, kind="ExternalOutput").ap()
              if debug_stop is not None else None)

    with tile.TileContext(nc) as tc:
        with (
            tc.tile_pool(name="big", bufs=1) as big,
            tc.tile_pool(name="wpool", bufs=1) as wpool,
            tc.tile_pool(name="mc", bufs=2) as mcp,
            tc.tile_pool(name="chk", bufs=2) as chk,
            tc.tile_pool(name="sm", bufs=1) as sm,
            tc.tile_pool(name="ps", bufs=3, space="PSUM") as psp,
            tc.tile_pool(name="ps1", bufs=1, space="PSUM") as psp1,
            tc.tile_pool(name="dram", bufs=1, space="DRAM") as drp,
        ):
            N1 = NPG[0]
            Z = big.tile([128, N1], BF16, tag="Z")
            BA = big.tile([128, N1], BF16, tag="BA")
            BB = big.tile([128, N1], BF16, tag="BB")

            # resident weights
            Wc = []
            for i in range(6):
                t = wpool.tile(list(dri[f"Wc{i}"].shape), BF16, tag=f"Wc{i}")
                nc.sync.dma_start(t[:], dri[f"Wc{i}"][:])
                Wc.append(t)
            gam = sm.tile([128, 5], F32, tag="gam")
            bet = sm.tile([128, 5], F32, tag="bet")
            ident = sm.tile([128, 128], BF16, tag="ident")
            negI = sm.tile([128, 128], BF16, tag="negI")
            neg2I = sm.tile([128, 128], BF16, tag="neg2I")
            nc.sync.dma_start(gam[:], dri["gam"][:])
            nc.sync.dma_start(bet[:], dri["bet"][:])
            nc.sync.dma_start(ident[:], dri["ident"][:])
            nc.sync.dma_start(negI[:], dri["negI"][:])
            nc.sync.dma_start(neg2I[:], dri["neg2I"][:])

            # resident M/C/diag for level 2 + o-weights for level 0
            resM = {}
            for lev in (2,):
                for nm_ in ("M", "Cup", "Cdn", "Df", "Db"):
                    sap = dri[f"{nm_}{lev}"]
                    t = wpool.tile([128, sap.shape[2]], BF16, tag=f"{nm_}{lev}")
                    nc.sync.dma_start(t[:], sap[0])
                    resM[(nm_, lev)] = t
            wo0 = wpool.tile([128, 2 * TILES[0]], BF16, tag="wo0")
            nc.sync.dma_start(wo0[:], dri["wo0"][:])

            # BN collective bounce
            bn_in = drp.tile([1, 2 * HID], F32)
            bn_out = drp.tile([1, 2 * HID], F32)

            # stats / bn vectors
            ONES = sm.tile([128, 1], BF16, tag="ONES")
            nc.vector.memset(ONES[:], 1.0)
            BN2 = sm.tile([1, 2 * HID], F32, tag="BN2")
            G2 = sm.tile([128, 2], F32, tag="G2")
            MEAN = sm.tile([128, 1], F32, tag="MEAN")
            VAR = sm.tile([128, 1], F32, tag="VAR")
            TMPV = sm.tile([128, 1], F32, tag="TMPV")
            Av = sm.tile([128, 1], F32, tag="Av")
            Cv = sm.tile([128, 1], F32, tag="Cv")

            def conv(ci):
                lev = CONV_LEV[ci]
                dinw, dout = CONV_DIN[ci], CONV_DOUT[ci]
                T = TILES[lev]
                tpl = TPL[lev]
                Ncols = T * dout
                compact = lev < 2
                cs = MC_CHUNK if compact else T
                ncol_c = LEV_S[lev] if compact else 128
                dcs = cs                           # tiles per DVE chunk
                zt = Z
                ndch = T // dcs
                b1, b2 = BA, BB                    # b1 = current b_{k+1}
                for k in range(5, -1, -1):
                    for c in range(ndch):
                        t0 = c * dcs
                        if ci == 0:
                            zch = mcp.tile([IN_D, dcs * 128], BF16, tag="zch")
                            nc.sync.dma_start(
                                zch[:], dri["xT"][:, t0 * 128:(t0 + dcs) * 128])
                        if k < 5:
                            if compact:
                                mt = mcp.tile([128, dcs * 128], BF16, tag="mt")
                                cu = mcp.tile([128, dcs * ncol_c], BF16, tag="cu")
                                cd = mcp.tile([128, dcs * ncol_c], BF16, tag="cd")
                                nc.sync.dma_start(mt[:], dri[f"M{lev}"][c])
                                nc.sync.dma_start(cu[:], dri[f"Cup{lev}"][c])
                                nc.sync.dma_start(cd[:], dri[f"Cdn{lev}"][c])
                                if lev > 0:
                                    df = mcp.tile([128, dcs * 128], BF16,
                                                  tag="df")
                                    db = mcp.tile([128, dcs * 128], BF16,
                                                  tag="db")
                                    nc.sync.dma_start(df[:], dri[f"Df{lev}"][c])
                                    nc.sync.dma_start(db[:], dri[f"Db{lev}"][c])
                            else:
                                mt = resM[("M", lev)]
                                cu = resM[("Cup", lev)]
                                cd = resM[("Cdn", lev)]
                                df = resM[("Df", lev)]
                                db = resM[("Db", lev)]
                        for gi in range(dcs // 4):
                            ps = psp.tile([128, 4 * dout], F32, tag="ps")
                            for ii in range(4):
                                t = t0 + gi * 4 + ii
                                tl = gi * 4 + ii
                                pslice = ps[:, ii * dout:(ii + 1) * dout]
                                wsl = Wc[ci][:, k * dout:(k + 1) * dout]
                                zsl = (zch[:, tl * 128:(tl + 1) * 128] if ci == 0
                                       else zt[:, t * 128:(t + 1) * 128])
                                mms = [dict(out=pslice, lhsT=zsl, rhs=wsl)]
                                if k == 0:
                                    mms.append(dict(out=pslice, lhsT=zsl, rhs=wsl))
                                if k < 5:
                                    if t > 0:
                                        mms.append(dict(
                                            out=pslice[0:ncol_c, :] if compact else pslice,
                                            lhsT=cu[:, tl * ncol_c:(tl + 1) * ncol_c],
                                            rhs=b1[:, (t - 1) * dout:t * dout]))
                                    if t + 1 < T:
                                        if compact:
                                            mms.append(dict(
                                                out=pslice[128 - ncol_c:128, :],
                                                lhsT=cd[:, tl * ncol_c:(tl + 1) * ncol_c],
                                                rhs=b1[:, (t + 1) * dout:(t + 2) * dout],
                                                tile_position=(0, 128 - ncol_c)))
                                        else:
                                            mms.append(dict(
                                                out=pslice,
                                                lhsT=cd[:, tl * 128:(tl + 1) * 128],
                                                rhs=b1[:, (t + 1) * dout:(t + 2) * dout]))
                                    if lev > 0:
                                        # o-ring: diagonal matrices into the
                                        # same PSUM accumulation
                                        srcf = (t - tpl) % T
                                        srcb = (t + tpl) % T
                                        mms.append(dict(
                                            out=pslice,
                                            lhsT=df[:, tl * 128:(tl + 1) * 128],
                                            rhs=b1[:, srcf * dout:(srcf + 1) * dout]))
                                        mms.append(dict(
                                            out=pslice,
                                            lhsT=db[:, tl * 128:(tl + 1) * 128],
                                            rhs=b1[:, srcb * dout:(srcb + 1) * dout]))
                                    if k <= 3:
                                        # -b_{k+2} (or -2*b_2 at k=0) via
                                        # negated identity, also in PSUM
                                        mms.append(dict(
                                            out=pslice,
                                            lhsT=(neg2I if k == 0 else negI)[:],
                                            rhs=b2[:, t * dout:(t + 1) * dout]))
                                    # full-partition M last so the group stop
                                    # covers every partition of the zero region
                                    mms.append(dict(
                                        out=pslice,
                                        lhsT=mt[:, tl * 128:(tl + 1) * 128],
                                        rhs=b1[:, t * dout:(t + 1) * dout]))
                                for mi, mm in enumerate(mms):
                                    nc.tensor.matmul(
                                        mm["out"], mm["lhsT"], mm["rhs"],
                                        start=(mi == 0), stop=(mi == len(mms) - 1),
                                        tile_position=mm.get("tile_position"))
                            # PSUM holds b_k (q=2*out at k=0); o-ring for
                            # level 0 is added below on the DVE
                            nc.scalar.copy(
                                b2[:, (t0 + gi * 4) * dout:(t0 + gi * 4 + 4) * dout],
                                ps[:])
                        if lev == 0 and k < 5:
                            # level-0 o-ring on DVE: bslice += w ⊙ b1[t∓tpl]
                            cc = dcs * dout
                            c0 = t0 * dout
                            bslice = b2[:, c0:c0 + cc]
                            u1 = chk.tile([128, dcs * dout], BF16, tag="u1")
                            for (s0, doff, n) in wrap_ranges(t0 - tpl, dcs, T):
                                wv = wo0[:, s0:s0 + n]
                                nc.vector.tensor_tensor(
                                    u1[:, doff * dout:(doff + n) * dout]
                                      .rearrange("p (t d) -> p t d", t=n),
                                    b1[:, s0 * dout:(s0 + n) * dout]
                                      .rearrange("p (t d) -> p t d", t=n),
                                    wv[:, :, None].broadcast_to([128, n, dout]),
                                    OP.mult)
                            nc.vector.tensor_tensor(bslice, bslice, u1[:],
                                                    OP.add)
                            u2 = chk.tile([128, dcs * dout], BF16, tag="u2")
                            for (s0, doff, n) in wrap_ranges(t0 + tpl, dcs, T):
                                wv = wo0[:, T + t0 + doff:T + t0 + doff + n]
                                nc.vector.tensor_tensor(
                                    u2[:, doff * dout:(doff + n) * dout]
                                      .rearrange("p (t d) -> p t d", t=n),
                                    b1[:, s0 * dout:(s0 + n) * dout]
                                      .rearrange("p (t d) -> p t d", t=n),
                                    wv[:, :, None].broadcast_to([128, n, dout]),
                                    OP.mult)
                            nc.vector.tensor_tensor(bslice, bslice, u2[:],
                                                    OP.add)
                    b1, b2 = b2, b1
                    if (isinstance(debug_stop, tuple) and debug_stop[0] == "b"
                            and debug_stop[1] == ci and debug_stop[2] == k):
                        nc.sync.dma_start(dbg_ap[:, 0:Ncols], b1[:, 0:Ncols])
                # q = 2p now lives in b1 (cols [0, Ncols))
                Q = b1

                if ci < 5:
                    # ---- BN stats on q: sums via PE ones-matmuls (PSUM
                    # accumulation over tiles), squares via ACT engine
                    bnp = psp1.tile([1, 2 * HID], F32, tag="bnps")
                    for t in range(T):
                        nc.tensor.matmul(bnp[:, 0:dout], ONES[:],
                                         Q[:, t * dout:(t + 1) * dout],
                                         start=(t == 0), stop=(t == T - 1))
                    nt_c = 8 if T % 8 == 0 else 4
                    cc = nt_c * dout
                    nch2 = Ncols // cc
                    for c in range(nch2):
                        sqt = chk.tile([128, 8 * dout], BF16, tag="sq")
                        nc.scalar.activation(sqt[:, 0:cc],
                                             Q[:, c * cc:(c + 1) * cc],
                                             AF.Square)
                        for tt in range(nt_c):
                            nc.tensor.matmul(
                                bnp[:, HID:HID + dout], ONES[:],
                                sqt[:, tt * dout:(tt + 1) * dout],
                                start=(c == 0 and tt == 0),
                                stop=(c == nch2 - 1 and tt == nt_c - 1))
                    nc.scalar.copy(BN2[:], bnp[:])
                    nc.sync.dma_start(bn_in[:], BN2[:])
                    if prof_nocoll:
                        nc.sync.dma_start(bn_out[:], bn_in[:])
                    else:
                        nc.gpsimd.collective_compute(
                            "AllReduce", OP.add,
                            replica_groups=[list(range(N_CORES))],
                            ins=[bn_in.opt()], outs=[bn_out.opt()])
                    nc.sync.dma_start(G2[:, 0:1], bn_out[0:1, 0:HID])
                    nc.sync.dma_start(G2[:, 1:2], bn_out[0:1, HID:2 * HID])
                    ntot = float(N_CORES * NPG[lev])
                    nc.vector.tensor_scalar_mul(MEAN[:], G2[:, 0:1], 1.0 / ntot)
                    nc.vector.tensor_scalar_mul(VAR[:], G2[:, 1:2], 1.0 / ntot)
                    nc.vector.tensor_tensor(TMPV[:], MEAN[:], MEAN[:], OP.mult)
                    nc.vector.tensor_tensor(VAR[:], VAR[:], TMPV[:], OP.subtract)
                    nc.vector.tensor_scalar_add(VAR[:], VAR[:], EPS2)
                    nc.scalar.sqrt(TMPV[:], VAR[:])
                    nc.vector.reciprocal(TMPV[:], TMPV[:])
                    nc.vector.tensor_tensor(Av[:], gam[:, ci:ci + 1], TMPV[:],
                                            OP.mult)
                    nc.vector.tensor_tensor(TMPV[:], Av[:], MEAN[:], OP.mult)
                    nc.vector.tensor_tensor(Cv[:], bet[:, ci:ci + 1], TMPV[:],
                                            OP.subtract)
                    if debug_stop == ("bn", ci):
                        BNDBG = sm.tile([128, 6], F32, tag="BNDBG")
                        nc.vector.tensor_copy(BNDBG[:, 0:1], G2[:, 0:1])
                        nc.vector.tensor_copy(BNDBG[:, 1:2], G2[:, 1:2])
                        nc.vector.tensor_copy(BNDBG[:, 2:3], MEAN[:])
                        nc.vector.tensor_copy(BNDBG[:, 3:4], VAR[:])
                        nc.vector.tensor_copy(BNDBG[:, 4:5], Av[:])
                        nc.vector.tensor_copy(BNDBG[:, 5:6], Cv[:])
                        BNB16 = sm.tile([128, 6], BF16, tag="BNB16")
                        nc.vector.tensor_copy(BNB16[:], BNDBG[:])
                        nc.sync.dma_start(dbg_ap[:, 0:6], BNB16[:])

                # ---- transpose to feat-major + fused BN-relu (or 0.5-relu)
                if ci < 5:
                    for gi in range(T // 4):
                        ps = psp.tile([128, 4 * 128], BF16, tag="tps")
                        for ii in range(4):
                            t = gi * 4 + ii
                            nc.tensor.transpose(
                                ps[:, ii * 128:(ii + 1) * 128],
                                Q[:, t * dout:(t + 1) * dout], ident[:])
                        nc.scalar.activation(
                            Z[:, gi * 512:(gi + 1) * 512], ps[:], AF.Relu,
                            bias=Cv[:], scale=Av[:])
                else:
                    Z6 = sm.tile([OUT_D, TILES[2] * 128], BF16, tag="Z6")
                    for gi in range(T // 4):
                        ps = psp.tile([128, 4 * 128], BF16, tag="tps")
                        for ii in range(4):
                            t = gi * 4 + ii
                            nc.tensor.transpose(
                                ps[0:OUT_D, ii * 128:(ii + 1) * 128],
                                Q[:, t * dout:(t + 1) * dout], ident[:])
                        nc.scalar.activation(
                            Z6[:, gi * 512:(gi + 1) * 512], ps[0:OUT_D, :],
                            AF.Relu, bias=0.0, scale=0.5)
                    return Z6

            def pool2x2(s, d=128):
                """Z [d, L*s*s] -> Z [d, L*(s/2)^2] via temp in BA."""
                n = L * s * s
                half = n // 2
                tmp = BA
                # x-pairs
                nc.vector.tensor_tensor(
                    tmp[0:d, 0:half],
                    Z[0:d, 0:n].rearrange("p (c two) -> p c two", two=2)[:, :, 0:1]
                      .rearrange("p c one -> p (c one)"),
                    Z[0:d, 0:n].rearrange("p (c two) -> p c two", two=2)[:, :, 1:2]
                      .rearrange("p c one -> p (c one)"),
                    OP.max)
                # y-pairs: cols (o, y, x2) with x2 = s/2
                x2 = s // 2
                v = tmp[0:d, 0:half].rearrange("p (o y x) -> p o y x", o=L, y=s)
                nc.vector.tensor_tensor(
                    Z[0:d, 0:half // 2].rearrange("p (o y x) -> p o y x",
                                                  o=L, y=s // 2),
                    v[:, :, 0::2, :], v[:, :, 1::2, :], OP.max)

            RES = sm.tile([1, OUT_D], F32, tag="RES")
            # ---------------- network ----------------
            def dbg_dump(si, buf, n):
                if debug_stop == si:
                    nc.sync.dma_start(dbg_ap[:, 0:n], buf[:, 0:n])

            if isinstance(debug_stop, tuple):
                dnum = -1
            else:
                dnum = debug_stop if isinstance(debug_stop, int) else 99
            conv(0)
            dbg_dump(0, Z, NPG[0])
            if dnum >= 1:
                conv(1)
                dbg_dump(1, Z, NPG[0])
            if dnum >= 2:
                pool2x2(S)
                dbg_dump(2, Z, NPG[1])
            if dnum >= 3:
                conv(2)
                dbg_dump(3, Z, NPG[1])
            if dnum >= 4:
                conv(3)
                dbg_dump(4, Z, NPG[1])
            if dnum >= 5:
                pool2x2(S // 2)
                dbg_dump(5, Z, NPG[2])
            if dnum >= 6:
                conv(4)
                dbg_dump(6, Z, NPG[2])
            Z6 = conv(5) if dnum >= 7 else None
            if Z6 is None:
                nc.vector.memset(RES[:], 0.0)
                nc.sync.dma_start(out_ap[:], RES[:])

            if Z6 is not None:
                s3 = S // 4
                n3 = L * s3 * s3
                P3 = sm.tile([OUT_D, n3 // 4], BF16, tag="P3")
                TMP3 = sm.tile([OUT_D, n3 // 2], BF16, tag="TMP3")
                nc.vector.tensor_tensor(
                    TMP3[:],
                    Z6[:].rearrange("p (c two) -> p c two", two=2)[:, :, 0:1]
                         .rearrange("p c one -> p (c one)"),
                    Z6[:].rearrange("p (c two) -> p c two", two=2)[:, :, 1:2]
                         .rearrange("p c one -> p (c one)"),
                    OP.max)
                v3 = TMP3[:].rearrange("p (o y x) -> p o y x", o=L, y=s3)
                nc.vector.tensor_tensor(
                    P3[:].rearrange("p (o y x) -> p o y x", o=L, y=s3 // 2),
                    v3[:, :, 0::2, :], v3[:, :, 1::2, :], OP.max)
                # orientation max over L slices of 64
                spp = (s3 // 2) * (s3 // 2)
                OM = sm.tile([OUT_D, spp], BF16, tag="OM")
                nc.vector.tensor_tensor(OM[:], P3[:, 0:spp], P3[:, spp:2 * spp],
                                        OP.max)
                for o in range(2, L):
                    nc.vector.tensor_tensor(OM[:], OM[:],
                                            P3[:, o * spp:(o + 1) * spp], OP.max)
                GV = sm.tile([OUT_D, 1], F32, tag="GV")
                nc.vector.tensor_reduce(GV[:], OM[:], mybir.AxisListType.X, OP.max)
                # -> [1, 10] via DRAM bounce
                gb_d = drp.tile([OUT_D, 1], F32)
                nc.sync.dma_start(gb_d[:], GV[:])
                GF = sm.tile([1, OUT_D], F32, tag="GF")
                nc.sync.dma_start(GF[:], gb_d[:].rearrange("a b -> b a"))
                M0 = sm.tile([1, 1], F32, tag="M0")
                nc.vector.tensor_reduce(M0[:], GF[:], mybir.AxisListType.X, OP.max)
                TD = sm.tile([1, OUT_D], F32, tag="TD")
                nc.vector.tensor_scalar(TD[:], GF[:], M0[:], None, OP.subtract)
                EX = sm.tile([1, OUT_D], F32, tag="EX")
                nc.scalar.activation(EX[:], TD[:], AF.Exp)
                SE = sm.tile([1, 1], F32, tag="SE")
                nc.vector.tensor_reduce(SE[:], EX[:], mybir.AxisListType.X, OP.add)
                LSE = sm.tile([1, 1], F32, tag="LSE")
                nc.scalar.activation(LSE[:], SE[:], AF.Ln)
                nc.vector.tensor_scalar(RES[:], TD[:], LSE[:], None, OP.subtract)
                nc.sync.dma_start(out_ap[:], RES[:])

    nc.compile()
    return nc


_CACHE = {}


def _get_nc():
    if "nc" not in _CACHE:
        _CACHE["nc"] = build_bass()
    return _CACHE["nc"]


def kernel(**inputs):
    nc = _get_nc()
    per_core = host_preprocess(inputs)
    res = run_bass_kernel_spmd(nc, per_core, list(range(N_CORES)))
    out = np.concatenate([res.results[c]["out"] for c in range(N_CORES)], axis=0)
    return out.astype(np.float32)



# revision 39
# speedup vs baseline: 28.3898x; 1.0767x over previous
"""ChebNet GNN forward on 8 Trainium2 NeuronCores — data-parallel over the 8 graphs.

The input graph is a structured 3D grid (orientation ring x spatial grid), so the
sparse ChebConv Laplacian becomes a 6-point stencil. Per ChebConv we evaluate the
K=6 Chebyshev sum with the Clenshaw recurrence:
    b_5 = c_5;  b_k = c_k + 2L b_{k+1} - b_{k+2};  out = c_0 + L b_1 - b_2
where c_k = z @ W_k. We actually produce q = 2*out; BatchNorm (applied with
eps' = 4*eps on q-statistics) absorbs the factor exactly; the final BN-less conv
applies 0.5 explicitly.

On-device layouts (per core = one graph):
  feat-major [d, N]  for conv inputs z (PE matmul contraction on features)
  node-major [128, T*dout] for Clenshaw states (tile t = 128 consecutive nodes)
Lap terms: x/y-neighbor stencil -> per-tile banded 128x128 matrices on the PE
(c_k and the in-tile/cross-tile products accumulate in PSUM); the orientation
ring (+-tiles_per_layer with wrap) runs on the DVE with compact per-node weights
broadcast along the feature axis via stride-0 APs. BN statistics are AllReduced
across the 8 cores.
"""

import numpy as np
import ml_dtypes

from concourse import bass, bacc, tile, mybir
from concourse.bass_utils import run_bass_kernel_spmd

BF16 = mybir.dt.bfloat16
F32 = mybir.dt.float32
AF = mybir.ActivationFunctionType
OP = mybir.AluOpType

B, S, L = 8, 64, 6
K = 6
IN_D, HID, OUT_D = 3, 128, 10
EPS2 = 4e-5
N_CORES = 8
LEV_S = [S, S // 2, S // 4]
NPG = [L * s * s for s in LEV_S]          # nodes per graph per level
TILES = [n // 128 for n in NPG]           # 192, 48, 12
TPL = [s * s // 128 for s in LEV_S]       # tiles per layer: 32, 8, 2
CONV_LEV = [0, 0, 1, 1, 2, 2]
CONV_DIN = [IN_D, HID, HID, HID, HID, HID]
CONV_DOUT = [HID, HID, HID, HID, HID, OUT_D]
MC_CHUNK = 8                               # tiles per streamed M/C chunk (levels 0-1)


def _bf(x):
    return np.asarray(x).astype(ml_dtypes.bfloat16)


# --------------------------------------------------------------------------
# host-side preprocessing (numpy)
# --------------------------------------------------------------------------

def parse_grid_weights(edge_index, edge_attr, s):
    src = edge_index[0].astype(np.int64)
    dst = edge_index[1].astype(np.int64)
    ea = np.asarray(edge_attr, np.float64)

    def coords(n):
        return n // (s * s * L), (n // (s * s)) % L, (n // s) % s, n % s

    bs, os_, ys, xs = coords(src)
    bd, od, yd, xd = coords(dst)
    g = {k: np.zeros((B, L, s, s), np.float64)
         for k in ("xf", "xb", "yf", "yb", "of", "ob")}
    same = bs == bd
    so = same & (os_ == od)
    m = so & (yd == ys) & (xd == xs + 1)
    np.add.at(g["xf"], (bs[m], os_[m], ys[m], xs[m]), ea[m])
    m = so & (yd == ys) & (xd == xs - 1)
    np.add.at(g["xb"], (bd[m], od[m], yd[m], xd[m]), ea[m])
    m = so & (xd == xs) & (yd == ys + 1)
    np.add.at(g["yf"], (bs[m], os_[m], ys[m], xs[m]), ea[m])
    m = so & (xd == xs) & (yd == ys - 1)
    np.add.at(g["yb"], (bd[m], od[m], yd[m], xd[m]), ea[m])
    m = same & (yd == ys) & (xd == xs) & (od == (os_ + 1) % L)
    np.add.at(g["of"], (bs[m], os_[m], ys[m], xs[m]), ea[m])
    m = same & (yd == ys) & (xd == xs) & (od == (os_ - 1) % L)
    np.add.at(g["ob"], (bd[m], od[m], yd[m], xd[m]), ea[m])
    return {k: v.astype(np.float32) for k, v in g.items()}


def build_level_mats(gb, s):
    """gb: one graph's grids [L,s,s]. Returns M [T,128,128], Cup [T,128,64|128],
    Cdn likewise, Df/Db [T,128,128] o-ring diagonals (all x2-baked)."""
    N = L * s * s
    T = N // 128
    R = 128 // s
    tpl = s * s // 128
    xf = gb["xf"].reshape(L * s, s)
    xb = gb["xb"].reshape(L * s, s)
    yf = gb["yf"].reshape(L * s, s)
    yb = gb["yb"].reshape(L * s, s)

    M = np.zeros((T, 128, 128), np.float32)
    Cup = np.zeros((T, 128, 128), np.float32)
    Cdn = np.zeros((T, 128, 128), np.float32)
    ar = np.arange(s - 1)
    ars = np.arange(s)
    for t in range(T):
        for r in range(R):
            row = t * R + r
            base = r * s
            M[t, base + ar, base + ar + 1] += 2 * xf[row, :-1]
            M[t, base + ar + 1, base + ar] += 2 * xb[row, :-1]
            if r + 1 < R:
                M[t, base + ars, base + s + ars] += 2 * yf[row]
                M[t, base + s + ars, base + ars] += 2 * yb[row]
        if t > 0:
            Cup[t, (R - 1) * s + ars, ars] = 2 * yf[(t - 1) * R + (R - 1)]
        if t + 1 < T:
            Cdn[t, ars, ars] = 2 * yb[t * R + (R - 1)]   # cols shifted to 0 (compact); device offsets out partitions
    # o-ring as per-tile diagonal matrices: dest tile t gets
    #   Df[t] @ b[(t-tpl)%T]  (forward edge, weight indexed at source)
    #   Db[t] @ b[(t+tpl)%T]  (backward edge, weight indexed at dest)
    wof_c = 2 * gb["of"].reshape(T, 128)        # [T(tile), 128(node)]
    wob_c = 2 * gb["ob"].reshape(T, 128)
    ai = np.arange(128)
    Df = np.zeros((T, 128, 128), np.float32)
    Db = np.zeros((T, 128, 128), np.float32)
    srcf = (np.arange(T) - tpl) % T
    Df[np.arange(T)[:, None], ai, ai] = wof_c[srcf]
    Db[np.arange(T)[:, None], ai, ai] = wob_c
    return (M, Cup, Cdn, Df, Db)


def pack_chunks(Mt, cs, ncols):
    """[T, 128, ncols] -> [nchunks, 128, cs*ncols] partition-major chunks."""
    T = Mt.shape[0]
    nch = (T + cs - 1) // cs
    out = np.zeros((nch, 128, cs * ncols), np.float32)
    for g in range(nch):
        blk = Mt[g * cs:(g + 1) * cs, :, :ncols]          # [<=cs, 128, ncols]
        n = blk.shape[0]
        out[g, :, :n * ncols] = blk.transpose(1, 0, 2).reshape(128, n * ncols)
    return out


def host_preprocess(inputs):
    """Returns list of 8 per-core input dicts + shared shapes info."""
    x = np.asarray(inputs["x"], np.float32)
    per_core = [dict() for _ in range(N_CORES)]
    for b in range(N_CORES):
        per_core[b]["xT"] = _bf(x.reshape(B, NPG[0], IN_D)[b].T.copy())

    for lev, s in enumerate(LEV_S):
        g = parse_grid_weights(np.asarray(inputs[f"edge_index{lev+1}"]),
                               np.asarray(inputs[f"edge_attr{lev+1}"]), s)
        compact = lev < 2
        ncol_c = s if compact else 128
        cs = MC_CHUNK if compact else TILES[lev]
        for b in range(N_CORES):
            gb = {k: v[b] for k, v in g.items()}
            M, Cup, Cdn, Df, Db = build_level_mats(gb, s)
            if not compact:
                # dense Cdn: move cols back to natural position (R-1)*s..127
                R = 128 // s
                Cd2 = np.zeros_like(Cdn)
                Cd2[:, :, (R - 1) * s:] = Cdn[:, :, :s]
                Cdn = Cd2
            per_core[b][f"M{lev}"] = _bf(pack_chunks(M, cs, 128))
            per_core[b][f"Cup{lev}"] = _bf(pack_chunks(Cup, cs, ncol_c))
            per_core[b][f"Cdn{lev}"] = _bf(pack_chunks(Cdn, cs, ncol_c))
            per_core[b][f"Df{lev}"] = _bf(pack_chunks(Df, cs, 128))
            per_core[b][f"Db{lev}"] = _bf(pack_chunks(Db, cs, 128))
            if lev < 2:
                # o-ring weights for DVE/Pool-routed chunks [128, 2T]
                wof = 2 * gb["of"].reshape(-1, 128).T
                wob = 2 * gb["ob"].reshape(-1, 128).T
                per_core[b][f"wo{lev}"] = _bf(
                    np.concatenate([wof, wob], axis=1).astype(np.float32))

    for i in range(6):
        Wk = np.asarray(inputs[f"W{i+1}"], np.float32)       # [K, din, dout]
        Wcat = np.concatenate([Wk[k] for k in range(K)], axis=1)  # [din, K*dout]
        for b in range(N_CORES):
            per_core[b][f"Wc{i}"] = _bf(Wcat)
    gam = np.stack([np.asarray(inputs[f"gamma{i+1}"], np.float32)
                    for i in range(5)], axis=1)              # [128, 5]
    bet = np.stack([np.asarray(inputs[f"beta{i+1}"], np.float32)
                    for i in range(5)], axis=1)
    ident = np.eye(128, dtype=np.float32)
    for b in range(N_CORES):
        per_core[b]["gam"] = gam
        per_core[b]["bet"] = bet
        per_core[b]["ident"] = _bf(ident)
        per_core[b]["negI"] = _bf(-ident)
        per_core[b]["neg2I"] = _bf(-2.0 * ident)
    return per_core


# --------------------------------------------------------------------------
# device kernel builder
# --------------------------------------------------------------------------

def wrap_ranges(t0, nt, T):
    """[(src_start, dst_offset, n), ...] for tiles (t0..t0+nt) mod T."""
    out = []
    done = 0
    while done < nt:
        s0 = (t0 + done) % T
        n = min(nt - done, T - s0)
        out.append((s0, done, n))
        done += n
    return out


def build_bass(debug_stop=None, prof_nocoll=False):
    nc = bacc.Bacc("TRN2", target_bir_lowering=False, debug=False,
                   num_devices=N_CORES)

    # ---- dram parameters
    dri = {}

    def din(name, shape, dt):
        dri[name] = nc.dram_tensor(name, shape, dt, kind="ExternalInput").ap()

    din("xT", [IN_D, NPG[0]], BF16)
    for lev in range(3):
        T = TILES[lev]
        cs = MC_CHUNK if lev < 2 else T
        nch = (T + cs - 1) // cs
        ncol_c = LEV_S[lev] if lev < 2 else 128
        din(f"M{lev}", [nch, 128, cs * 128], BF16)
        din(f"Cup{lev}", [nch, 128, cs * ncol_c], BF16)
        din(f"Cdn{lev}", [nch, 128, cs * ncol_c], BF16)
        din(f"Df{lev}", [nch, 128, cs * 128], BF16)
        din(f"Db{lev}", [nch, 128, cs * 128], BF16)
        if lev < 2:
            din(f"wo{lev}", [128, 2 * T], BF16)
    din("Wc0", [IN_D, K * HID], BF16)
    for i in range(1, 5):
        din(f"Wc{i}", [HID, K * HID], BF16)
    din("Wc5", [HID, K * OUT_D], BF16)
    din("gam", [128, 5], F32)
    din("bet", [128, 5], F32)
    din("ident", [128, 128], BF16)
    din("negI", [128, 128], BF16)
    din("neg2I", [128, 128], BF16)
    out_ap = nc.dram_tensor("out", [1, OUT_D], F32, kind="ExternalOutput").ap()
    dbg_ap = (nc.dram_tensor("dbg", [128, NPG[0]], BF16, kind="ExternalOutput").ap()
              if debug_stop is not None else None)

    with tile.TileContext(nc) as tc:
        with (
            tc.tile_pool(name="big", bufs=1) as big,
            tc.tile_pool(name="wpool", bufs=1) as wpool,
            tc.tile_pool(name="mc", bufs=2) as mcp,
            tc.tile_pool(name="chk", bufs=2) as chk,
            tc.tile_pool(name="sm", bufs=1) as sm,
            tc.tile_pool(name="ps", bufs=3, space="PSUM") as psp,
            tc.tile_pool(name="ps1", bufs=1, space="PSUM") as psp1,
            tc.tile_pool(name="dram", bufs=1, space="DRAM") as drp,
        ):
            N1 = NPG[0]
            Z = big.tile([128, N1], BF16, tag="Z")
            BA = big.tile([128, N1], BF16, tag="BA")
            BB = big.tile([128, N1], BF16, tag="BB")

            # resident weights
            Wc = []
            for i in range(6):
                t = wpool.tile(list(dri[f"Wc{i}"].shape), BF16, tag=f"Wc{i}")
                nc.sync.dma_start(t[:], dri[f"Wc{i}"][:])
                Wc.append(t)
            gam = sm.tile([128, 5], F32, tag="gam")
            bet = sm.tile([128, 5], F32, tag="bet")
            ident = sm.tile([128, 128], BF16, tag="ident")
            negI = sm.tile([128, 128], BF16, tag="negI")
            neg2I = sm.tile([128, 128], BF16, tag="neg2I")
            nc.sync.dma_start(gam[:], dri["gam"][:])
            nc.sync.dma_start(bet[:], dri["bet"][:])
            nc.sync.dma_start(ident[:], dri["ident"][:])
            nc.sync.dma_start(negI[:], dri["negI"][:])
            nc.sync.dma_start(neg2I[:], dri["neg2I"][:])

            # resident M/C/diag for level 2 + o-weights for level 0
            resM = {}
            for lev in (2,):
                for nm_ in ("M", "Cup", "Cdn", "Df", "Db"):
                    sap = dri[f"{nm_}{lev}"]
                    t = wpool.tile([128, sap.shape[2]], BF16, tag=f"{nm_}{lev}")
                    nc.sync.dma_start(t[:], sap[0])
                    resM[(nm_, lev)] = t
            wo = {}
            for lev in (0, 1):
                wot = wpool.tile([128, 2 * TILES[lev]], BF16, tag=f"wo{lev}")
                nc.sync.dma_start(wot[:], dri[f"wo{lev}"][:])
                wo[lev] = wot

            # BN collective bounce
            bn_in = drp.tile([1, 2 * HID], F32)
            bn_out = drp.tile([1, 2 * HID], F32)

            # stats / bn vectors
            ONES = sm.tile([128, 1], BF16, tag="ONES")
            nc.vector.memset(ONES[:], 1.0)
            BN2 = sm.tile([1, 2 * HID], F32, tag="BN2")
            G2 = sm.tile([128, 2], F32, tag="G2")
            MEAN = sm.tile([128, 1], F32, tag="MEAN")
            VAR = sm.tile([128, 1], F32, tag="VAR")
            TMPV = sm.tile([128, 1], F32, tag="TMPV")
            Av = sm.tile([128, 1], F32, tag="Av")
            Cv = sm.tile([128, 1], F32, tag="Cv")

            def conv(ci):
                lev = CONV_LEV[ci]
                dinw, dout = CONV_DIN[ci], CONV_DOUT[ci]
                T = TILES[lev]
                tpl = TPL[lev]
                Ncols = T * dout
                compact = lev < 2
                cs = MC_CHUNK if compact else T
                ncol_c = LEV_S[lev] if compact else 128
                dcs = cs                           # tiles per DVE chunk
                zt = Z
                ndch = T // dcs
                b1, b2 = BA, BB                    # b1 = current b_{k+1}
                for k in range(5, -1, -1):
                    for c in range(ndch):
                        t0 = c * dcs
                        # o-ring route for this chunk: PE (streamed diag
                        # matmuls), Pool, or DVE (broadcast mult+add)
                        if lev == 0:
                            route = ("pe" if c % 4 == 1 else
                                     "pool" if c % 4 == 3 else "dve")
                        elif lev == 1:
                            route = "pe" if c % 2 == 0 else "dve"
                        else:
                            route = "pe"
                        if ci == 0:
                            zch = mcp.tile([IN_D, dcs * 128], BF16, tag="zch")
                            nc.sync.dma_start(
                                zch[:], dri["xT"][:, t0 * 128:(t0 + dcs) * 128])
                        if k < 5:
                            if compact:
                                mt = mcp.tile([128, dcs * 128], BF16, tag="mt")
                                cu = mcp.tile([128, dcs * ncol_c], BF16, tag="cu")
                                cd = mcp.tile([128, dcs * ncol_c], BF16, tag="cd")
                                nc.sync.dma_start(mt[:], dri[f"M{lev}"][c])
                                nc.sync.dma_start(cu[:], dri[f"Cup{lev}"][c])
                                nc.sync.dma_start(cd[:], dri[f"Cdn{lev}"][c])
                                if route == "pe":
                                    df = mcp.tile([128, dcs * 128], BF16,
                                                  tag="df")
                                    db = mcp.tile([128, dcs * 128], BF16,
                                                  tag="db")
                                    nc.sync.dma_start(df[:], dri[f"Df{lev}"][c])
                                    nc.sync.dma_start(db[:], dri[f"Db{lev}"][c])
                            else:
                                mt = resM[("M", lev)]
                                cu = resM[("Cup", lev)]
                                cd = resM[("Cdn", lev)]
                                df = resM[("Df", lev)]
                                db = resM[("Db", lev)]
                        for gi in range(dcs // 4):
                            ps = psp.tile([128, 4 * dout], F32, tag="ps")
                            for ii in range(4):
                                t = t0 + gi * 4 + ii
                                tl = gi * 4 + ii
                                pslice = ps[:, ii * dout:(ii + 1) * dout]
                                wsl = Wc[ci][:, k * dout:(k + 1) * dout]
                                zsl = (zch[:, tl * 128:(tl + 1) * 128] if ci == 0
                                       else zt[:, t * 128:(t + 1) * 128])
                                mms = [dict(out=pslice, lhsT=zsl, rhs=wsl)]
                                if k == 0:
                                    mms.append(dict(out=pslice, lhsT=zsl, rhs=wsl))
                                if k < 5:
                                    if t > 0:
                                        mms.append(dict(
                                            out=pslice[0:ncol_c, :] if compact else pslice,
                                            lhsT=cu[:, tl * ncol_c:(tl + 1) * ncol_c],
                                            rhs=b1[:, (t - 1) * dout:t * dout]))
                                    if t + 1 < T:
                                        if compact:
                                            mms.append(dict(
                                                out=pslice[128 - ncol_c:128, :],
                                                lhsT=cd[:, tl * ncol_c:(tl + 1) * ncol_c],
                                                rhs=b1[:, (t + 1) * dout:(t + 2) * dout],
                                                tile_position=(0, 128 - ncol_c)))
                                        else:
                                            mms.append(dict(
                                                out=pslice,
                                                lhsT=cd[:, tl * 128:(tl + 1) * 128],
                                                rhs=b1[:, (t + 1) * dout:(t + 2) * dout]))
                                    if route == "pe":
                                        # o-ring: diagonal matrices into the
                                        # same PSUM accumulation
                                        srcf = (t - tpl) % T
                                        srcb = (t + tpl) % T
                                        mms.append(dict(
                                            out=pslice,
                                            lhsT=df[:, tl * 128:(tl + 1) * 128],
                                            rhs=b1[:, srcf * dout:(srcf + 1) * dout]))
                                        mms.append(dict(
                                            out=pslice,
                                            lhsT=db[:, tl * 128:(tl + 1) * 128],
                                            rhs=b1[:, srcb * dout:(srcb + 1) * dout]))
                                    if k <= 3:
                                        # -b_{k+2} (or -2*b_2 at k=0) via
                                        # negated identity, also in PSUM
                                        mms.append(dict(
                                            out=pslice,
                                            lhsT=(neg2I if k == 0 else negI)[:],
                                            rhs=b2[:, t * dout:(t + 1) * dout]))
                                    # full-partition M last so the group stop
                                    # covers every partition of the zero region
                                    mms.append(dict(
                                        out=pslice,
                                        lhsT=mt[:, tl * 128:(tl + 1) * 128],
                                        rhs=b1[:, t * dout:(t + 1) * dout]))
                                for mi, mm in enumerate(mms):
                                    nc.tensor.matmul(
                                        mm["out"], mm["lhsT"], mm["rhs"],
                                        start=(mi == 0), stop=(mi == len(mms) - 1),
                                        tile_position=mm.get("tile_position"))
                            # PSUM holds b_k (q=2*out at k=0); o-ring for
                            # level 0 is added below on the DVE
                            nc.scalar.copy(
                                b2[:, (t0 + gi * 4) * dout:(t0 + gi * 4 + 4) * dout],
                                ps[:])
                        if route != "pe" and k < 5:
                            # o-ring on DVE or Pool: bslice += w ⊙ b1[t∓tpl]
                            eng = nc.vector if route == "dve" else nc.gpsimd
                            sfx = "" if route == "dve" else "p"
                            cc = dcs * dout
                            c0 = t0 * dout
                            bslice = b2[:, c0:c0 + cc]
                            u1 = chk.tile([128, dcs * dout], BF16,
                                          tag="u1" + sfx)
                            for (s0, doff, n) in wrap_ranges(t0 - tpl, dcs, T):
                                wv = wo[lev][:, s0:s0 + n]
                                eng.tensor_tensor(
                                    u1[:, doff * dout:(doff + n) * dout]
                                      .rearrange("p (t d) -> p t d", t=n),
                                    b1[:, s0 * dout:(s0 + n) * dout]
                                      .rearrange("p (t d) -> p t d", t=n),
                                    wv[:, :, None].broadcast_to([128, n, dout]),
                                    OP.mult)
                            eng.tensor_tensor(bslice, bslice, u1[:], OP.add)
                            u2 = chk.tile([128, dcs * dout], BF16,
                                          tag="u2" if route == "dve" else "u1p")
                            for (s0, doff, n) in wrap_ranges(t0 + tpl, dcs, T):
                                wv = wo[lev][:, T + t0 + doff:T + t0 + doff + n]
                                eng.tensor_tensor(
                                    u2[:, doff * dout:(doff + n) * dout]
                                      .rearrange("p (t d) -> p t d", t=n),
                                    b1[:, s0 * dout:(s0 + n) * dout]
                                      .rearrange("p (t d) -> p t d", t=n),
                                    wv[:, :, None].broadcast_to([128, n, dout]),
                                    OP.mult)
                            eng.tensor_tensor(bslice, bslice, u2[:], OP.add)
                    b1, b2 = b2, b1
                    if (isinstance(debug_stop, tuple) and debug_stop[0] == "b"
                            and debug_stop[1] == ci and debug_stop[2] == k):
                        nc.sync.dma_start(dbg_ap[:, 0:Ncols], b1[:, 0:Ncols])
                # q = 2p now lives in b1 (cols [0, Ncols))
                Q = b1

                if ci < 5:
                    # ---- BN stats on q: sums via PE ones-matmuls (PSUM
                    # accumulation over tiles), squares via ACT engine
                    bnp = psp1.tile([1, 2 * HID], F32, tag="bnps")
                    for t in range(T):
                        nc.tensor.matmul(bnp[:, 0:dout], ONES[:],
                                         Q[:, t * dout:(t + 1) * dout],
                                         start=(t == 0), stop=(t == T - 1))
                    nt_c = 8 if T % 8 == 0 else 4
                    cc = nt_c * dout
                    nch2 = Ncols // cc
                    for c in range(nch2):
                        sqt = chk.tile([128, 8 * dout], BF16, tag="u1")
                        nc.scalar.activation(sqt[:, 0:cc],
                                             Q[:, c * cc:(c + 1) * cc],
                                             AF.Square)
                        for tt in range(nt_c):
                            nc.tensor.matmul(
                                bnp[:, HID:HID + dout], ONES[:],
                                sqt[:, tt * dout:(tt + 1) * dout],
                                start=(c == 0 and tt == 0),
                                stop=(c == nch2 - 1 and tt == nt_c - 1))
                    nc.scalar.copy(BN2[:], bnp[:])
                    nc.sync.dma_start(bn_in[:], BN2[:])
                    if prof_nocoll:
                        nc.sync.dma_start(bn_out[:], bn_in[:])
                    else:
                        nc.gpsimd.collective_compute(
                            "AllReduce", OP.add,
                            replica_groups=[list(range(N_CORES))],
                            ins=[bn_in.opt()], outs=[bn_out.opt()])
                    nc.sync.dma_start(G2[:, 0:1], bn_out[0:1, 0:HID])
                    nc.sync.dma_start(G2[:, 1:2], bn_out[0:1, HID:2 * HID])
                    ntot = float(N_CORES * NPG[lev])
                    nc.vector.tensor_scalar_mul(MEAN[:], G2[:, 0:1], 1.0 / ntot)
                    nc.vector.tensor_scalar_mul(VAR[:], G2[:, 1:2], 1.0 / ntot)
                    nc.vector.tensor_tensor(TMPV[:], MEAN[:], MEAN[:], OP.mult)
                    nc.vector.tensor_tensor(VAR[:], VAR[:], TMPV[:], OP.subtract)
                    nc.vector.tensor_scalar_add(VAR[:], VAR[:], EPS2)
                    nc.scalar.sqrt(TMPV[:], VAR[:])
                    nc.vector.reciprocal(TMPV[:], TMPV[:])
                    nc.vector.tensor_tensor(Av[:], gam[:, ci:ci + 1], TMPV[:],
                                            OP.mult)
                    nc.vector.tensor_tensor(TMPV[:], Av[:], MEAN[:], OP.mult)
                    nc.vector.tensor_tensor(Cv[:], bet[:, ci:ci + 1], TMPV[:],
                                            OP.subtract)
                    if debug_stop == ("bn", ci):
                        BNDBG = sm.tile([128, 6], F32, tag="BNDBG")
                        nc.vector.tensor_copy(BNDBG[:, 0:1], G2[:, 0:1])
                        nc.vector.tensor_copy(BNDBG[:, 1:2], G2[:, 1:2])
                        nc.vector.tensor_copy(BNDBG[:, 2:3], MEAN[:])
                        nc.vector.tensor_copy(BNDBG[:, 3:4], VAR[:])
                        nc.vector.tensor_copy(BNDBG[:, 4:5], Av[:])
                        nc.vector.tensor_copy(BNDBG[:, 5:6], Cv[:])
                        BNB16 = sm.tile([128, 6], BF16, tag="BNB16")
                        nc.vector.tensor_copy(BNB16[:], BNDBG[:])
                        nc.sync.dma_start(dbg_ap[:, 0:6], BNB16[:])

                # ---- transpose to feat-major + fused BN-relu (or 0.5-relu)
                if ci < 5:
                    for gi in range(T // 4):
                        ps = psp.tile([128, 4 * 128], BF16, tag="tps")
                        for ii in range(4):
                            t = gi * 4 + ii
                            nc.tensor.transpose(
                                ps[:, ii * 128:(ii + 1) * 128],
                                Q[:, t * dout:(t + 1) * dout], ident[:])
                        nc.scalar.activation(
                            Z[:, gi * 512:(gi + 1) * 512], ps[:], AF.Relu,
                            bias=Cv[:], scale=Av[:])
                else:
                    Z6 = sm.tile([OUT_D, TILES[2] * 128], BF16, tag="Z6")
                    for gi in range(T // 4):
                        ps = psp.tile([128, 4 * 128], BF16, tag="tps")
                        for ii in range(4):
                            t = gi * 4 + ii
                            nc.tensor.transpose(
                                ps[0:OUT_D, ii * 128:(ii + 1) * 128],
                                Q[:, t * dout:(t + 1) * dout], ident[:])
                        nc.scalar.activation(
                            Z6[:, gi * 512:(gi + 1) * 512], ps[0:OUT_D, :],
                            AF.Relu, bias=0.0, scale=0.5)
                    return Z6

            def pool2x2(s, d=128):
                """Z [d, L*s*s] -> Z [d, L*(s/2)^2] via temp in BA."""
                n = L * s * s
                half = n // 2
                tmp = BA
                # x-pairs
                nc.vector.tensor_tensor(
                    tmp[0:d, 0:half],
                    Z[0:d, 0:n].rearrange("p (c two) -> p c two", two=2)[:, :, 0:1]
                      .rearrange("p c one -> p (c one)"),
                    Z[0:d, 0:n].rearrange("p (c two) -> p c two", two=2)[:, :, 1:2]
                      .rearrange("p c one -> p (c one)"),
                    OP.max)
                # y-pairs: cols (o, y, x2) with x2 = s/2
                x2 = s // 2
                v = tmp[0:d, 0:half].rearrange("p (o y x) -> p o y x", o=L, y=s)
                nc.vector.tensor_tensor(
                    Z[0:d, 0:half // 2].rearrange("p (o y x) -> p o y x",
                                                  o=L, y=s // 2),
                    v[:, :, 0::2, :], v[:, :, 1::2, :], OP.max)

            RES = sm.tile([1, OUT_D], F32, tag="RES")
            # ---------------- network ----------------
            def dbg_dump(si, buf, n):
                if debug_stop == si:
                    nc.sync.dma_start(dbg_ap[:, 0:n], buf[:, 0:n])

            if isinstance(debug_stop, tuple):
                dnum = -1
            else:
                dnum = debug_stop if isinstance(debug_stop, int) else 99
            conv(0)
            dbg_dump(0, Z, NPG[0])
            if dnum >= 1:
                conv(1)
                dbg_dump(1, Z, NPG[0])
            if dnum >= 2:
                pool2x2(S)
                dbg_dump(2, Z, NPG[1])
            if dnum >= 3:
                conv(2)
                dbg_dump(3, Z, NPG[1])
            if dnum >= 4:
                conv(3)
                dbg_dump(4, Z, NPG[1])
            if dnum >= 5:
                pool2x2(S // 2)
                dbg_dump(5, Z, NPG[2])
            if dnum >= 6:
                conv(4)
                dbg_dump(6, Z, NPG[2])
            Z6 = conv(5) if dnum >= 7 else None
            if Z6 is None:
                nc.vector.memset(RES[:], 0.0)
                nc.sync.dma_start(out_ap[:], RES[:])

            if Z6 is not None:
                s3 = S // 4
                n3 = L * s3 * s3
                P3 = sm.tile([OUT_D, n3 // 4], BF16, tag="P3")
                TMP3 = sm.tile([OUT_D, n3 // 2], BF16, tag="TMP3")
                nc.vector.tensor_tensor(
                    TMP3[:],
                    Z6[:].rearrange("p (c two) -> p c two", two=2)[:, :, 0:1]
                         .rearrange("p c one -> p (c one)"),
                    Z6[:].rearrange("p (c two) -> p c two", two=2)[:, :, 1:2]
                         .rearrange("p c one -> p (c one)"),
                    OP.max)
                v3 = TMP3[:].rearrange("p (o y x) -> p o y x", o=L, y=s3)
                nc.vector.tensor_tensor(
                    P3[:].rearrange("p (o y x) -> p o y x", o=L, y=s3 // 2),
                    v3[:, :, 0::2, :], v3[:, :, 1::2, :], OP.max)
                # orientation max over L slices of 64
                spp = (s3 // 2) * (s3 // 2)
                OM = sm.tile([OUT_D, spp], BF16, tag="OM")
                nc.vector.tensor_tensor(OM[:], P3[:, 0:spp], P3[:, spp:2 * spp],
                                        OP.max)
                for o in range(2, L):
                    nc.vector.tensor_tensor(OM[:], OM[:],
                                            P3[:, o * spp:(o + 1) * spp], OP.max)
                GV = sm.tile([OUT_D, 1], F32, tag="GV")
                nc.vector.tensor_reduce(GV[:], OM[:], mybir.AxisListType.X, OP.max)
                # -> [1, 10] via DRAM bounce
                gb_d = drp.tile([OUT_D, 1], F32)
                nc.sync.dma_start(gb_d[:], GV[:])
                GF = sm.tile([1, OUT_D], F32, tag="GF")
                nc.sync.dma_start(GF[:], gb_d[:].rearrange("a b -> b a"))
                M0 = sm.tile([1, 1], F32, tag="M0")
                nc.vector.tensor_reduce(M0[:], GF[:], mybir.AxisListType.X, OP.max)
                TD = sm.tile([1, OUT_D], F32, tag="TD")
                nc.vector.tensor_scalar(TD[:], GF[:], M0[:], None, OP.subtract)
                EX = sm.tile([1, OUT_D], F32, tag="EX")
                nc.scalar.activation(EX[:], TD[:], AF.Exp)
                SE = sm.tile([1, 1], F32, tag="SE")
                nc.vector.tensor_reduce(SE[:], EX[:], mybir.AxisListType.X, OP.add)
                LSE = sm.tile([1, 1], F32, tag="LSE")
                nc.scalar.activation(LSE[:], SE[:], AF.Ln)
                nc.vector.tensor_scalar(RES[:], TD[:], LSE[:], None, OP.subtract)
                nc.sync.dma_start(out_ap[:], RES[:])

    nc.compile()
    return nc


_CACHE = {}


def _get_nc():
    if "nc" not in _CACHE:
        _CACHE["nc"] = build_bass()
    return _CACHE["nc"]


def kernel(**inputs):
    nc = _get_nc()
    per_core = host_preprocess(inputs)
    res = run_bass_kernel_spmd(nc, per_core, list(range(N_CORES)))
    out = np.concatenate([res.results[c]["out"] for c in range(N_CORES)], axis=0)
    return out.astype(np.float32)



# revision 40
# speedup vs baseline: 29.2223x; 1.0293x over previous
"""ChebNet GNN forward on 8 Trainium2 NeuronCores — data-parallel over the 8 graphs.

The input graph is a structured 3D grid (orientation ring x spatial grid), so the
sparse ChebConv Laplacian becomes a 6-point stencil. Per ChebConv we evaluate the
K=6 Chebyshev sum with the Clenshaw recurrence:
    b_5 = c_5;  b_k = c_k + 2L b_{k+1} - b_{k+2};  out = c_0 + L b_1 - b_2
where c_k = z @ W_k. We actually produce q = 2*out; BatchNorm (applied with
eps' = 4*eps on q-statistics) absorbs the factor exactly; the final BN-less conv
applies 0.5 explicitly.

On-device layouts (per core = one graph):
  feat-major [d, N]  for conv inputs z (PE matmul contraction on features)
  node-major [128, T*dout] for Clenshaw states (tile t = 128 consecutive nodes)
Lap terms: x/y-neighbor stencil -> per-tile banded 128x128 matrices on the PE
(c_k and the in-tile/cross-tile products accumulate in PSUM); the orientation
ring (+-tiles_per_layer with wrap) runs on the DVE with compact per-node weights
broadcast along the feature axis via stride-0 APs. BN statistics are AllReduced
across the 8 cores.
"""

import numpy as np
import ml_dtypes

from concourse import bass, bacc, tile, mybir
from concourse.bass_utils import run_bass_kernel_spmd

BF16 = mybir.dt.bfloat16
F32 = mybir.dt.float32
AF = mybir.ActivationFunctionType
OP = mybir.AluOpType

B, S, L = 8, 64, 6
K = 6
IN_D, HID, OUT_D = 3, 128, 10
EPS2 = 4e-5
N_CORES = 8
LEV_S = [S, S // 2, S // 4]
NPG = [L * s * s for s in LEV_S]          # nodes per graph per level
TILES = [n // 128 for n in NPG]           # 192, 48, 12
TPL = [s * s // 128 for s in LEV_S]       # tiles per layer: 32, 8, 2
CONV_LEV = [0, 0, 1, 1, 2, 2]
CONV_DIN = [IN_D, HID, HID, HID, HID, HID]
CONV_DOUT = [HID, HID, HID, HID, HID, OUT_D]
MC_CHUNK = 8                               # tiles per streamed M/C chunk (levels 0-1)


def _bf(x):
    return np.asarray(x).astype(ml_dtypes.bfloat16)


# --------------------------------------------------------------------------
# host-side preprocessing (numpy)
# --------------------------------------------------------------------------

def parse_grid_weights(edge_index, edge_attr, s):
    src = edge_index[0].astype(np.int64)
    dst = edge_index[1].astype(np.int64)
    ea = np.asarray(edge_attr, np.float64)

    def coords(n):
        return n // (s * s * L), (n // (s * s)) % L, (n // s) % s, n % s

    bs, os_, ys, xs = coords(src)
    bd, od, yd, xd = coords(dst)
    g = {k: np.zeros((B, L, s, s), np.float64)
         for k in ("xf", "xb", "yf", "yb", "of", "ob")}
    same = bs == bd
    so = same & (os_ == od)
    m = so & (yd == ys) & (xd == xs + 1)
    np.add.at(g["xf"], (bs[m], os_[m], ys[m], xs[m]), ea[m])
    m = so & (yd == ys) & (xd == xs - 1)
    np.add.at(g["xb"], (bd[m], od[m], yd[m], xd[m]), ea[m])
    m = so & (xd == xs) & (yd == ys + 1)
    np.add.at(g["yf"], (bs[m], os_[m], ys[m], xs[m]), ea[m])
    m = so & (xd == xs) & (yd == ys - 1)
    np.add.at(g["yb"], (bd[m], od[m], yd[m], xd[m]), ea[m])
    m = same & (yd == ys) & (xd == xs) & (od == (os_ + 1) % L)
    np.add.at(g["of"], (bs[m], os_[m], ys[m], xs[m]), ea[m])
    m = same & (yd == ys) & (xd == xs) & (od == (os_ - 1) % L)
    np.add.at(g["ob"], (bd[m], od[m], yd[m], xd[m]), ea[m])
    return {k: v.astype(np.float32) for k, v in g.items()}


def build_level_mats(gb, s):
    """gb: one graph's grids [L,s,s]. Returns M [T,128,128], Cup [T,128,64|128],
    Cdn likewise, Df/Db [T,128,128] o-ring diagonals (all x2-baked)."""
    N = L * s * s
    T = N // 128
    R = 128 // s
    tpl = s * s // 128
    xf = gb["xf"].reshape(L * s, s)
    xb = gb["xb"].reshape(L * s, s)
    yf = gb["yf"].reshape(L * s, s)
    yb = gb["yb"].reshape(L * s, s)

    M = np.zeros((T, 128, 128), np.float32)
    Cup = np.zeros((T, 128, 128), np.float32)
    Cdn = np.zeros((T, 128, 128), np.float32)
    ar = np.arange(s - 1)
    ars = np.arange(s)
    for t in range(T):
        for r in range(R):
            row = t * R + r
            base = r * s
            M[t, base + ar, base + ar + 1] += 2 * xf[row, :-1]
            M[t, base + ar + 1, base + ar] += 2 * xb[row, :-1]
            if r + 1 < R:
                M[t, base + ars, base + s + ars] += 2 * yf[row]
                M[t, base + s + ars, base + ars] += 2 * yb[row]
        if t > 0:
            Cup[t, (R - 1) * s + ars, ars] = 2 * yf[(t - 1) * R + (R - 1)]
        if t + 1 < T:
            Cdn[t, ars, ars] = 2 * yb[t * R + (R - 1)]   # cols shifted to 0 (compact); device offsets out partitions
    # o-ring as per-tile diagonal matrices: dest tile t gets
    #   Df[t] @ b[(t-tpl)%T]  (forward edge, weight indexed at source)
    #   Db[t] @ b[(t+tpl)%T]  (backward edge, weight indexed at dest)
    wof_c = 2 * gb["of"].reshape(T, 128)        # [T(tile), 128(node)]
    wob_c = 2 * gb["ob"].reshape(T, 128)
    ai = np.arange(128)
    Df = np.zeros((T, 128, 128), np.float32)
    Db = np.zeros((T, 128, 128), np.float32)
    srcf = (np.arange(T) - tpl) % T
    Df[np.arange(T)[:, None], ai, ai] = wof_c[srcf]
    Db[np.arange(T)[:, None], ai, ai] = wob_c
    return (M, Cup, Cdn, Df, Db)


def pack_chunks(Mt, cs, ncols):
    """[T, 128, ncols] -> [nchunks, 128, cs*ncols] partition-major chunks."""
    T = Mt.shape[0]
    nch = (T + cs - 1) // cs
    out = np.zeros((nch, 128, cs * ncols), np.float32)
    for g in range(nch):
        blk = Mt[g * cs:(g + 1) * cs, :, :ncols]          # [<=cs, 128, ncols]
        n = blk.shape[0]
        out[g, :, :n * ncols] = blk.transpose(1, 0, 2).reshape(128, n * ncols)
    return out


def host_preprocess(inputs):
    """Returns list of 8 per-core input dicts + shared shapes info."""
    x = np.asarray(inputs["x"], np.float32)
    per_core = [dict() for _ in range(N_CORES)]
    for b in range(N_CORES):
        per_core[b]["xT"] = _bf(x.reshape(B, NPG[0], IN_D)[b].T.copy())

    for lev, s in enumerate(LEV_S):
        g = parse_grid_weights(np.asarray(inputs[f"edge_index{lev+1}"]),
                               np.asarray(inputs[f"edge_attr{lev+1}"]), s)
        compact = lev < 2
        ncol_c = s if compact else 128
        cs = MC_CHUNK if compact else TILES[lev]
        for b in range(N_CORES):
            gb = {k: v[b] for k, v in g.items()}
            M, Cup, Cdn, Df, Db = build_level_mats(gb, s)
            if not compact:
                # dense Cdn: move cols back to natural position (R-1)*s..127
                R = 128 // s
                Cd2 = np.zeros_like(Cdn)
                Cd2[:, :, (R - 1) * s:] = Cdn[:, :, :s]
                Cdn = Cd2
            per_core[b][f"M{lev}"] = _bf(pack_chunks(M, cs, 128))
            per_core[b][f"Cup{lev}"] = _bf(pack_chunks(Cup, cs, ncol_c))
            per_core[b][f"Cdn{lev}"] = _bf(pack_chunks(Cdn, cs, ncol_c))
            per_core[b][f"Df{lev}"] = _bf(pack_chunks(Df, cs, 128))
            per_core[b][f"Db{lev}"] = _bf(pack_chunks(Db, cs, 128))
            if lev < 2:
                # o-ring weights for DVE/Pool-routed chunks [128, 2T]
                wof = 2 * gb["of"].reshape(-1, 128).T
                wob = 2 * gb["ob"].reshape(-1, 128).T
                per_core[b][f"wo{lev}"] = _bf(
                    np.concatenate([wof, wob], axis=1).astype(np.float32))

    for i in range(6):
        Wk = np.asarray(inputs[f"W{i+1}"], np.float32)       # [K, din, dout]
        Wcat = np.concatenate([Wk[k] for k in range(K)], axis=1)  # [din, K*dout]
        for b in range(N_CORES):
            per_core[b][f"Wc{i}"] = _bf(Wcat)
    gam = np.stack([np.asarray(inputs[f"gamma{i+1}"], np.float32)
                    for i in range(5)], axis=1)              # [128, 5]
    bet = np.stack([np.asarray(inputs[f"beta{i+1}"], np.float32)
                    for i in range(5)], axis=1)
    ident = np.eye(128, dtype=np.float32)
    for b in range(N_CORES):
        per_core[b]["gam"] = gam
        per_core[b]["bet"] = bet
        per_core[b]["ident"] = _bf(ident)
        per_core[b]["negI"] = _bf(-ident)
        per_core[b]["neg2I"] = _bf(-2.0 * ident)
    return per_core


# --------------------------------------------------------------------------
# device kernel builder
# --------------------------------------------------------------------------

def wrap_ranges(t0, nt, T):
    """[(src_start, dst_offset, n), ...] for tiles (t0..t0+nt) mod T."""
    out = []
    done = 0
    while done < nt:
        s0 = (t0 + done) % T
        n = min(nt - done, T - s0)
        out.append((s0, done, n))
        done += n
    return out


def build_bass(debug_stop=None, prof_nocoll=False):
    nc = bacc.Bacc("TRN2", target_bir_lowering=False, debug=False,
                   num_devices=N_CORES)

    # ---- dram parameters
    dri = {}

    def din(name, shape, dt):
        dri[name] = nc.dram_tensor(name, shape, dt, kind="ExternalInput").ap()

    din("xT", [IN_D, NPG[0]], BF16)
    for lev in range(3):
        T = TILES[lev]
        cs = MC_CHUNK if lev < 2 else T
        nch = (T + cs - 1) // cs
        ncol_c = LEV_S[lev] if lev < 2 else 128
        din(f"M{lev}", [nch, 128, cs * 128], BF16)
        din(f"Cup{lev}", [nch, 128, cs * ncol_c], BF16)
        din(f"Cdn{lev}", [nch, 128, cs * ncol_c], BF16)
        din(f"Df{lev}", [nch, 128, cs * 128], BF16)
        din(f"Db{lev}", [nch, 128, cs * 128], BF16)
        if lev < 2:
            din(f"wo{lev}", [128, 2 * T], BF16)
    din("Wc0", [IN_D, K * HID], BF16)
    for i in range(1, 5):
        din(f"Wc{i}", [HID, K * HID], BF16)
    din("Wc5", [HID, K * OUT_D], BF16)
    din("gam", [128, 5], F32)
    din("bet", [128, 5], F32)
    din("ident", [128, 128], BF16)
    din("negI", [128, 128], BF16)
    din("neg2I", [128, 128], BF16)
    out_ap = nc.dram_tensor("out", [1, OUT_D], F32, kind="ExternalOutput").ap()
    dbg_ap = (nc.dram_tensor("dbg", [128, NPG[0]], BF16, kind="ExternalOutput").ap()
              if debug_stop is not None else None)

    with tile.TileContext(nc) as tc:
        with (
            tc.tile_pool(name="big", bufs=1) as big,
            tc.tile_pool(name="wpool", bufs=1) as wpool,
            tc.tile_pool(name="mc", bufs=2) as mcp,
            tc.tile_pool(name="chk", bufs=2) as chk,
            tc.tile_pool(name="sm", bufs=1) as sm,
            tc.tile_pool(name="ps", bufs=3, space="PSUM") as psp,
            tc.tile_pool(name="ps1", bufs=1, space="PSUM") as psp1,
            tc.tile_pool(name="dram", bufs=1, space="DRAM") as drp,
        ):
            N1 = NPG[0]
            Z = big.tile([128, N1], BF16, tag="Z")
            BA = big.tile([128, N1], BF16, tag="BA")
            BB = big.tile([128, N1], BF16, tag="BB")

            # resident weights
            Wc = []
            for i in range(6):
                t = wpool.tile(list(dri[f"Wc{i}"].shape), BF16, tag=f"Wc{i}")
                nc.sync.dma_start(t[:], dri[f"Wc{i}"][:])
                Wc.append(t)
            gam = sm.tile([128, 5], F32, tag="gam")
            bet = sm.tile([128, 5], F32, tag="bet")
            ident = sm.tile([128, 128], BF16, tag="ident")
            negI = sm.tile([128, 128], BF16, tag="negI")
            neg2I = sm.tile([128, 128], BF16, tag="neg2I")
            nc.sync.dma_start(gam[:], dri["gam"][:])
            nc.sync.dma_start(bet[:], dri["bet"][:])
            nc.sync.dma_start(ident[:], dri["ident"][:])
            nc.sync.dma_start(negI[:], dri["negI"][:])
            nc.sync.dma_start(neg2I[:], dri["neg2I"][:])

            # resident M/C/diag for level 2 + o-weights for level 0
            resM = {}
            for lev in (2,):
                for nm_ in ("M", "Cup", "Cdn", "Df", "Db"):
                    sap = dri[f"{nm_}{lev}"]
                    t = wpool.tile([128, sap.shape[2]], BF16, tag=f"{nm_}{lev}")
                    nc.sync.dma_start(t[:], sap[0])
                    resM[(nm_, lev)] = t
            wo = {}
            for lev in (0, 1):
                wot = wpool.tile([128, 2 * TILES[lev]], BF16, tag=f"wo{lev}")
                nc.sync.dma_start(wot[:], dri[f"wo{lev}"][:])
                wo[lev] = wot

            # BN collective bounce
            bn_in = drp.tile([1, 2 * HID], F32)
            bn_out = drp.tile([1, 2 * HID], F32)

            # stats / bn vectors
            ONES = sm.tile([128, 1], BF16, tag="ONES")
            nc.vector.memset(ONES[:], 1.0)
            BN2 = sm.tile([1, 2 * HID], F32, tag="BN2")
            G2 = sm.tile([128, 2], F32, tag="G2")
            MEAN = sm.tile([128, 1], F32, tag="MEAN")
            VAR = sm.tile([128, 1], F32, tag="VAR")
            TMPV = sm.tile([128, 1], F32, tag="TMPV")
            Av = sm.tile([128, 1], F32, tag="Av")
            Cv = sm.tile([128, 1], F32, tag="Cv")

            def conv(ci):
                lev = CONV_LEV[ci]
                dinw, dout = CONV_DIN[ci], CONV_DOUT[ci]
                T = TILES[lev]
                tpl = TPL[lev]
                Ncols = T * dout
                compact = lev < 2
                cs = MC_CHUNK if compact else T
                ncol_c = LEV_S[lev] if compact else 128
                dcs = cs                           # tiles per DVE chunk
                zt = Z
                ndch = T // dcs
                b1, b2 = BA, BB                    # b1 = current b_{k+1}
                for k in range(5, -1, -1):
                    for c in range(ndch):
                        t0 = c * dcs
                        # o-ring route for this chunk: PE (streamed diag
                        # matmuls), Pool, or DVE (broadcast mult+add)
                        if lev == 0:
                            # 6 PE / 5 Pool / 13 DVE: evens the per-k phase
                            # times (Pool is ~2.5x slower per chunk than DVE)
                            route = ("pe" if c % 4 == 1 else
                                     "pool" if (c % 4 == 3 and c != 7)
                                     else "dve")
                        elif lev == 1:
                            route = "pe" if c % 2 == 0 else "dve"
                        else:
                            route = "pe"
                        if ci == 0:
                            zch = mcp.tile([IN_D, dcs * 128], BF16, tag="zch")
                            nc.sync.dma_start(
                                zch[:], dri["xT"][:, t0 * 128:(t0 + dcs) * 128])
                        if k < 5:
                            if compact:
                                mt = mcp.tile([128, dcs * 128], BF16, tag="mt")
                                cu = mcp.tile([128, dcs * ncol_c], BF16, tag="cu")
                                cd = mcp.tile([128, dcs * ncol_c], BF16, tag="cd")
                                nc.sync.dma_start(mt[:], dri[f"M{lev}"][c])
                                nc.sync.dma_start(cu[:], dri[f"Cup{lev}"][c])
                                nc.sync.dma_start(cd[:], dri[f"Cdn{lev}"][c])
                                if route == "pe":
                                    df = mcp.tile([128, dcs * 128], BF16,
                                                  tag="df")
                                    db = mcp.tile([128, dcs * 128], BF16,
                                                  tag="db")
                                    nc.sync.dma_start(df[:], dri[f"Df{lev}"][c])
                                    nc.sync.dma_start(db[:], dri[f"Db{lev}"][c])
                            else:
                                mt = resM[("M", lev)]
                                cu = resM[("Cup", lev)]
                                cd = resM[("Cdn", lev)]
                                df = resM[("Df", lev)]
                                db = resM[("Db", lev)]
                        for gi in range(dcs // 4):
                            ps = psp.tile([128, 4 * dout], F32, tag="ps")
                            for ii in range(4):
                                t = t0 + gi * 4 + ii
                                tl = gi * 4 + ii
                                pslice = ps[:, ii * dout:(ii + 1) * dout]
                                wsl = Wc[ci][:, k * dout:(k + 1) * dout]
                                zsl = (zch[:, tl * 128:(tl + 1) * 128] if ci == 0
                                       else zt[:, t * 128:(t + 1) * 128])
                                mms = [dict(out=pslice, lhsT=zsl, rhs=wsl)]
                                if k == 0:
                                    mms.append(dict(out=pslice, lhsT=zsl, rhs=wsl))
                                if k < 5:
                                    if t > 0:
                                        mms.append(dict(
                                            out=pslice[0:ncol_c, :] if compact else pslice,
                                            lhsT=cu[:, tl * ncol_c:(tl + 1) * ncol_c],
                                            rhs=b1[:, (t - 1) * dout:t * dout]))
                                    if t + 1 < T:
                                        if compact:
                                            mms.append(dict(
                                                out=pslice[128 - ncol_c:128, :],
                                                lhsT=cd[:, tl * ncol_c:(tl + 1) * ncol_c],
                                                rhs=b1[:, (t + 1) * dout:(t + 2) * dout],
                                                tile_position=(0, 128 - ncol_c)))
                                        else:
                                            mms.append(dict(
                                                out=pslice,
                                                lhsT=cd[:, tl * 128:(tl + 1) * 128],
                                                rhs=b1[:, (t + 1) * dout:(t + 2) * dout]))
                                    if route == "pe":
                                        # o-ring: diagonal matrices into the
                                        # same PSUM accumulation
                                        srcf = (t - tpl) % T
                                        srcb = (t + tpl) % T
                                        mms.append(dict(
                                            out=pslice,
                                            lhsT=df[:, tl * 128:(tl + 1) * 128],
                                            rhs=b1[:, srcf * dout:(srcf + 1) * dout]))
                                        mms.append(dict(
                                            out=pslice,
                                            lhsT=db[:, tl * 128:(tl + 1) * 128],
                                            rhs=b1[:, srcb * dout:(srcb + 1) * dout]))
                                    if k <= 3:
                                        # -b_{k+2} (or -2*b_2 at k=0) via
                                        # negated identity, also in PSUM
                                        mms.append(dict(
                                            out=pslice,
                                            lhsT=(neg2I if k == 0 else negI)[:],
                                            rhs=b2[:, t * dout:(t + 1) * dout]))
                                    # full-partition M last so the group stop
                                    # covers every partition of the zero region
                                    mms.append(dict(
                                        out=pslice,
                                        lhsT=mt[:, tl * 128:(tl + 1) * 128],
                                        rhs=b1[:, t * dout:(t + 1) * dout]))
                                for mi, mm in enumerate(mms):
                                    nc.tensor.matmul(
                                        mm["out"], mm["lhsT"], mm["rhs"],
                                        start=(mi == 0), stop=(mi == len(mms) - 1),
                                        tile_position=mm.get("tile_position"))
                            # PSUM holds b_k (q=2*out at k=0); o-ring for
                            # level 0 is added below on the DVE
                            nc.scalar.copy(
                                b2[:, (t0 + gi * 4) * dout:(t0 + gi * 4 + 4) * dout],
                                ps[:])
                        if route != "pe" and k < 5:
                            # o-ring on DVE or Pool: bslice += w ⊙ b1[t∓tpl]
                            eng = nc.vector if route == "dve" else nc.gpsimd
                            sfx = "" if route == "dve" else "p"
                            cc = dcs * dout
                            c0 = t0 * dout
                            bslice = b2[:, c0:c0 + cc]
                            u1 = chk.tile([128, dcs * dout], BF16,
                                          tag="u1" + sfx)
                            for (s0, doff, n) in wrap_ranges(t0 - tpl, dcs, T):
                                wv = wo[lev][:, s0:s0 + n]
                                eng.tensor_tensor(
                                    u1[:, doff * dout:(doff + n) * dout]
                                      .rearrange("p (t d) -> p t d", t=n),
                                    b1[:, s0 * dout:(s0 + n) * dout]
                                      .rearrange("p (t d) -> p t d", t=n),
                                    wv[:, :, None].broadcast_to([128, n, dout]),
                                    OP.mult)
                            eng.tensor_tensor(bslice, bslice, u1[:], OP.add)
                            u2 = chk.tile([128, dcs * dout], BF16,
                                          tag="u2" if route == "dve" else "u1p")
                            for (s0, doff, n) in wrap_ranges(t0 + tpl, dcs, T):
                                wv = wo[lev][:, T + t0 + doff:T + t0 + doff + n]
                                eng.tensor_tensor(
                                    u2[:, doff * dout:(doff + n) * dout]
                                      .rearrange("p (t d) -> p t d", t=n),
                                    b1[:, s0 * dout:(s0 + n) * dout]
                                      .rearrange("p (t d) -> p t d", t=n),
                                    wv[:, :, None].broadcast_to([128, n, dout]),
                                    OP.mult)
                            eng.tensor_tensor(bslice, bslice, u2[:], OP.add)
                    b1, b2 = b2, b1
                    if (isinstance(debug_stop, tuple) and debug_stop[0] == "b"
                            and debug_stop[1] == ci and debug_stop[2] == k):
                        nc.sync.dma_start(dbg_ap[:, 0:Ncols], b1[:, 0:Ncols])
                # q = 2p now lives in b1 (cols [0, Ncols))
                Q = b1

                if ci < 5:
                    # ---- BN stats on q: sums via PE ones-matmuls (PSUM
                    # accumulation over tiles), squares via ACT engine
                    bnp = psp1.tile([1, 2 * HID], F32, tag="bnps")
                    for t in range(T):
                        nc.tensor.matmul(bnp[:, 0:dout], ONES[:],
                                         Q[:, t * dout:(t + 1) * dout],
                                         start=(t == 0), stop=(t == T - 1))
                    nt_c = 8 if T % 8 == 0 else 4
                    cc = nt_c * dout
                    nch2 = Ncols // cc
                    for c in range(nch2):
                        sqt = chk.tile([128, 8 * dout], BF16, tag="u1")
                        nc.scalar.activation(sqt[:, 0:cc],
                                             Q[:, c * cc:(c + 1) * cc],
                                             AF.Square)
                        for tt in range(nt_c):
                            nc.tensor.matmul(
                                bnp[:, HID:HID + dout], ONES[:],
                                sqt[:, tt * dout:(tt + 1) * dout],
                                start=(c == 0 and tt == 0),
                                stop=(c == nch2 - 1 and tt == nt_c - 1))
                    nc.scalar.copy(BN2[:], bnp[:])
                    nc.sync.dma_start(bn_in[:], BN2[:])
                    if prof_nocoll:
                        nc.sync.dma_start(bn_out[:], bn_in[:])
                    else:
                        nc.gpsimd.collective_compute(
                            "AllReduce", OP.add,
                            replica_groups=[list(range(N_CORES))],
                            ins=[bn_in.opt()], outs=[bn_out.opt()])
                    nc.sync.dma_start(G2[:, 0:1], bn_out[0:1, 0:HID])
                    nc.sync.dma_start(G2[:, 1:2], bn_out[0:1, HID:2 * HID])
                    ntot = float(N_CORES * NPG[lev])
                    nc.vector.tensor_scalar_mul(MEAN[:], G2[:, 0:1], 1.0 / ntot)
                    nc.vector.tensor_scalar_mul(VAR[:], G2[:, 1:2], 1.0 / ntot)
                    nc.vector.tensor_tensor(TMPV[:], MEAN[:], MEAN[:], OP.mult)
                    nc.vector.tensor_tensor(VAR[:], VAR[:], TMPV[:], OP.subtract)
                    nc.vector.tensor_scalar_add(VAR[:], VAR[:], EPS2)
                    nc.scalar.sqrt(TMPV[:], VAR[:])
                    nc.vector.reciprocal(TMPV[:], TMPV[:])
                    nc.vector.tensor_tensor(Av[:], gam[:, ci:ci + 1], TMPV[:],
                                            OP.mult)
                    nc.vector.tensor_tensor(TMPV[:], Av[:], MEAN[:], OP.mult)
                    nc.vector.tensor_tensor(Cv[:], bet[:, ci:ci + 1], TMPV[:],
                                            OP.subtract)
                    if debug_stop == ("bn", ci):
                        BNDBG = sm.tile([128, 6], F32, tag="BNDBG")
                        nc.vector.tensor_copy(BNDBG[:, 0:1], G2[:, 0:1])
                        nc.vector.tensor_copy(BNDBG[:, 1:2], G2[:, 1:2])
                        nc.vector.tensor_copy(BNDBG[:, 2:3], MEAN[:])
                        nc.vector.tensor_copy(BNDBG[:, 3:4], VAR[:])
                        nc.vector.tensor_copy(BNDBG[:, 4:5], Av[:])
                        nc.vector.tensor_copy(BNDBG[:, 5:6], Cv[:])
                        BNB16 = sm.tile([128, 6], BF16, tag="BNB16")
                        nc.vector.tensor_copy(BNB16[:], BNDBG[:])
                        nc.sync.dma_start(dbg_ap[:, 0:6], BNB16[:])

                # ---- transpose to feat-major + fused BN-relu (or 0.5-relu)
                if ci < 5:
                    for gi in range(T // 4):
                        ps = psp.tile([128, 4 * 128], BF16, tag="tps")
                        for ii in range(4):
                            t = gi * 4 + ii
                            nc.tensor.transpose(
                                ps[:, ii * 128:(ii + 1) * 128],
                                Q[:, t * dout:(t + 1) * dout], ident[:])
                        nc.scalar.activation(
                            Z[:, gi * 512:(gi + 1) * 512], ps[:], AF.Relu,
                            bias=Cv[:], scale=Av[:])
                else:
                    Z6 = sm.tile([OUT_D, TILES[2] * 128], BF16, tag="Z6")
                    for gi in range(T // 4):
                        ps = psp.tile([128, 4 * 128], BF16, tag="tps")
                        for ii in range(4):
                            t = gi * 4 + ii
                            nc.tensor.transpose(
                                ps[0:OUT_D, ii * 128:(ii + 1) * 128],
                                Q[:, t * dout:(t + 1) * dout], ident[:])
                        nc.scalar.activation(
                            Z6[:, gi * 512:(gi + 1) * 512], ps[0:OUT_D, :],
                            AF.Relu, bias=0.0, scale=0.5)
                    return Z6

            def pool2x2(s, d=128):
                """Z [d, L*s*s] -> Z [d, L*(s/2)^2] via temp in BA."""
                n = L * s * s
                half = n // 2
                tmp = BA
                # x-pairs
                nc.vector.tensor_tensor(
                    tmp[0:d, 0:half],
                    Z[0:d, 0:n].rearrange("p (c two) -> p c two", two=2)[:, :, 0:1]
                      .rearrange("p c one -> p (c one)"),
                    Z[0:d, 0:n].rearrange("p (c two) -> p c two", two=2)[:, :, 1:2]
                      .rearrange("p c one -> p (c one)"),
                    OP.max)
                # y-pairs: cols (o, y, x2) with x2 = s/2
                x2 = s // 2
                v = tmp[0:d, 0:half].rearrange("p (o y x) -> p o y x", o=L, y=s)
                nc.vector.tensor_tensor(
                    Z[0:d, 0:half // 2].rearrange("p (o y x) -> p o y x",
                                                  o=L, y=s // 2),
                    v[:, :, 0::2, :], v[:, :, 1::2, :], OP.max)

            RES = sm.tile([1, OUT_D], F32, tag="RES")
            # ---------------- network ----------------
            def dbg_dump(si, buf, n):
                if debug_stop == si:
                    nc.sync.dma_start(dbg_ap[:, 0:n], buf[:, 0:n])

            if isinstance(debug_stop, tuple):
                dnum = -1
            else:
                dnum = debug_stop if isinstance(debug_stop, int) else 99
            conv(0)
            dbg_dump(0, Z, NPG[0])
            if dnum >= 1:
                conv(1)
                dbg_dump(1, Z, NPG[0])
            if dnum >= 2:
                pool2x2(S)
                dbg_dump(2, Z, NPG[1])
            if dnum >= 3:
                conv(2)
                dbg_dump(3, Z, NPG[1])
            if dnum >= 4:
                conv(3)
                dbg_dump(4, Z, NPG[1])
            if dnum >= 5:
                pool2x2(S // 2)
                dbg_dump(5, Z, NPG[2])
            if dnum >= 6:
                conv(4)
                dbg_dump(6, Z, NPG[2])
            Z6 = conv(5) if dnum >= 7 else None
            if Z6 is None:
                nc.vector.memset(RES[:], 0.0)
                nc.sync.dma_start(out_ap[:], RES[:])

            if Z6 is not None:
                s3 = S // 4
                n3 = L * s3 * s3
                P3 = sm.tile([OUT_D, n3 // 4], BF16, tag="P3")
                TMP3 = sm.tile([OUT_D, n3 // 2], BF16, tag="TMP3")
                nc.vector.tensor_tensor(
                    TMP3[:],
                    Z6[:].rearrange("p (c two) -> p c two", two=2)[:, :, 0:1]
                         .rearrange("p c one -> p (c one)"),
                    Z6[:].rearrange("p (c two) -> p c two", two=2)[:, :, 1:2]
                         .rearrange("p c one -> p (c one)"),
                    OP.max)
                v3 = TMP3[:].rearrange("p (o y x) -> p o y x", o=L, y=s3)
                nc.vector.tensor_tensor(
                    P3[:].rearrange("p (o y x) -> p o y x", o=L, y=s3 // 2),
                    v3[:, :, 0::2, :], v3[:, :, 1::2, :], OP.max)
                # orientation max over L slices of 64
                spp = (s3 // 2) * (s3 // 2)
                OM = sm.tile([OUT_D, spp], BF16, tag="OM")
                nc.vector.tensor_tensor(OM[:], P3[:, 0:spp], P3[:, spp:2 * spp],
                                        OP.max)
                for o in range(2, L):
                    nc.vector.tensor_tensor(OM[:], OM[:],
                                            P3[:, o * spp:(o + 1) * spp], OP.max)
                GV = sm.tile([OUT_D, 1], F32, tag="GV")
                nc.vector.tensor_reduce(GV[:], OM[:], mybir.AxisListType.X, OP.max)
                # -> [1, 10] via DRAM bounce
                gb_d = drp.tile([OUT_D, 1], F32)
                nc.sync.dma_start(gb_d[:], GV[:])
                GF = sm.tile([1, OUT_D], F32, tag="GF")
                nc.sync.dma_start(GF[:], gb_d[:].rearrange("a b -> b a"))
                M0 = sm.tile([1, 1], F32, tag="M0")
                nc.vector.tensor_reduce(M0[:], GF[:], mybir.AxisListType.X, OP.max)
                TD = sm.tile([1, OUT_D], F32, tag="TD")
                nc.vector.tensor_scalar(TD[:], GF[:], M0[:], None, OP.subtract)
                EX = sm.tile([1, OUT_D], F32, tag="EX")
                nc.scalar.activation(EX[:], TD[:], AF.Exp)
                SE = sm.tile([1, 1], F32, tag="SE")
                nc.vector.tensor_reduce(SE[:], EX[:], mybir.AxisListType.X, OP.add)
                LSE = sm.tile([1, 1], F32, tag="LSE")
                nc.scalar.activation(LSE[:], SE[:], AF.Ln)
                nc.vector.tensor_scalar(RES[:], TD[:], LSE[:], None, OP.subtract)
                nc.sync.dma_start(out_ap[:], RES[:])

    nc.compile()
    return nc


_CACHE = {}


def _get_nc():
    if "nc" not in _CACHE:
        _CACHE["nc"] = build_bass()
    return _CACHE["nc"]


def kernel(**inputs):
    nc = _get_nc()
    per_core = host_preprocess(inputs)
    res = run_bass_kernel_spmd(nc, per_core, list(range(N_CORES)))
    out = np.concatenate([res.results[c]["out"] for c in range(N_CORES)], axis=0)
    return out.astype(np.float32)

